# revision 1
# baseline (speedup 1.0000x reference)
"""GAT kernel builder for TRN2 (8-core SPMD, dst-sharded ELL layout).

Design:
- Nodes padded to NP = 8*SH; core c owns dst/node rows [c*SH, (c+1)*SH).
- Per core, dsts are degree-sorted (ascending, pads first); edges stored in
  an ELL slot grid per 128-dst tile: grid[:, off_t + j] = src id of slot j
  (pad slots -> TRASH node NP-1, whose a_src is poisoned to -1e30 so its
  exp(lrelu(...)) underflows to exactly 0).
- K_t (slots per tile) equalized across cores (SPMD: one program).
- Gather: one indirect_dma_start per slot column (128 rows, one per
  partition = dst). f32 end-to-end.
- Layer tables: T1 [NP,72] = [h1(64) | a_src1(8)]; a_dst1 kept per-shard.
  T2 [NP,17] = [h2(16) | a_src2(1)]; a_dst2 per-shard. AllGather between
  layers via collective.
"""
import numpy as np

import concourse.bacc as bacc
import concourse.bass as bass
import concourse.mybir as mybir
import concourse.tile as tile

F32 = mybir.dt.float32
I32 = mybir.dt.int32
AF = mybir.ActivationFunctionType
OP = mybir.AluOpType

NEG_SLOPE = 0.2
EPS = 1e-16
POISON = -1.0e30


# ---------------------------------------------------------------- host prep

def host_prep(x, edge_index, W1, att_src1, att_dst1, b1, W2, att_src2,
              att_dst2, b2, n_cores=8):
    """Pure index/layout prep on host. Returns (in_maps, meta)."""
    N = x.shape[0]
    F_IN = x.shape[1]
    H1, C1 = att_src1.shape
    C2 = att_src2.shape[1]
    SH = -(-N // (128 * n_cores)) * 128          # shard rows, mult of 128
    NP = SH * n_cores
    T = SH // 128                                 # dst tiles per core
    TRASH = NP - 1

    src = np.concatenate([np.asarray(edge_index[0]), np.arange(N)]).astype(np.int64)
    dst = np.concatenate([np.asarray(edge_index[1]), np.arange(N)]).astype(np.int64)

    # per-core CSR by dst
    core_of = dst // SH
    perms = []            # [n_cores][128, T] int32  natural local row of sorted pos
    grids_per_core = []   # [n_cores][T] list of [128, K_t] arrays (pre-equalize)
    Ks = np.zeros((n_cores, T), dtype=np.int64)
    deg_sorted_idx = []
    for c in range(n_cores):
        m = core_of == c
        s_c = src[m]
        d_loc = (dst[m] - c * SH).astype(np.int64)
        deg = np.bincount(d_loc, minlength=SH)
        order = np.argsort(deg, kind="stable")    # ascending; zero-degree pads first
        deg_sorted_idx.append(order)
        # CSR over local dst
        sort_by_d = np.argsort(d_loc, kind="stable")
        s_sorted = s_c[sort_by_d]
        rowptr = np.zeros(SH + 1, dtype=np.int64)
        np.cumsum(deg, out=rowptr[1:])
        perms.append(order)
        tiles = []
        for t in range(T):
            dts = order[t * 128:(t + 1) * 128]
            K_t = max(int(deg[dts].max()), 1)
            Ks[c, t] = K_t
            g = np.full((128, K_t), TRASH, dtype=np.int32)
            for p, dl in enumerate(dts):
                a, b = rowptr[dl], rowptr[dl + 1]
                g[p, : b - a] = s_sorted[a:b]
            tiles.append(g)
        grids_per_core.append(tiles)

    # equalize K_t across cores
    K_eq = Ks.max(axis=0).astype(np.int64)        # [T]
    offs = np.zeros(T + 1, dtype=np.int64)
    np.cumsum(K_eq, out=offs[1:])
    GK = int(offs[-1])
    grids = []
    for c in range(n_cores):
        g_all = np.full((128, GK), TRASH, dtype=np.int32)
        for t in range(T):
            g = grids_per_core[c][t]
            g_all[:, offs[t]:offs[t] + g.shape[1]] = g
        grids.append(g_all)

    # global position map: node n -> its AllGather row (cores emit sorted shards)
    posmap = np.zeros(NP, dtype=np.int32)
    for c in range(n_cores):
        inv = np.empty(SH, dtype=np.int64)
        inv[perms[c]] = np.arange(SH)
        posmap[c * SH:(c + 1) * SH] = (c * SH + inv).astype(np.int32)
    grids = [posmap[g] for g in grids]
    trash_pos = int(posmap[NP - 1])

    # x transposed + padded; per-core columns in SORTED order
    xT = np.zeros((F_IN, NP), dtype=np.float32)
    xT[:, :N] = np.asarray(x, dtype=np.float32).T

    # weight prep (block-diag fold of attention vectors into the projection)
    W1 = np.asarray(W1, np.float32)
    W2 = np.asarray(W2, np.float32)
    BDs = np.zeros((H1 * C1, H1), np.float32)
    BDd = np.zeros((H1 * C1, H1), np.float32)
    for h in range(H1):
        BDs[h * C1:(h + 1) * C1, h] = np.asarray(att_src1, np.float32)[h]
        BDd[h * C1:(h + 1) * C1, h] = np.asarray(att_dst1, np.float32)[h]
    W1ext = np.concatenate([W1, W1 @ BDs, W1 @ BDd], axis=1)          # [F_IN, 80]
    W2ext = np.concatenate(
        [W2, W2 @ np.asarray(att_src2, np.float32).reshape(-1, 1),
         W2 @ np.asarray(att_dst2, np.float32).reshape(-1, 1)], axis=1)  # [64, 18]

    ident = np.eye(128, dtype=np.float32)

    in_maps = []
    for c in range(n_cores):
        in_maps.append({
            "xT_shard": np.ascontiguousarray(xT[:, c * SH + perms[c]]),
            "W1ext": W1ext,
            "W2ext": W2ext,
            "b1v": np.asarray(b1, np.float32).reshape(1, -1),
            "b2v": np.asarray(b2, np.float32).reshape(1, -1),
            "grid": grids[c],
            "ident": ident,
        })
    meta = dict(N=N, NP=NP, SH=SH, T=T, GK=GK, K_eq=K_eq.tolist(),
                offs=offs.tolist(), F_IN=F_IN, H1=H1, C1=C1, C2=C2,
                n_cores=n_cores, trash_pos=trash_pos)
    return in_maps, meta, perms


# ------------------------------------------------------------- device build

def build_program(meta, repeat=1):
    NP, SH, T, GK = meta["NP"], meta["SH"], meta["T"], meta["GK"]
    F_IN = meta["F_IN"]
    H1, C1, C2 = meta["H1"], meta["C1"], meta["C2"]
    D1 = H1 * C1                   # 64
    R1 = D1 + H1                   # 72  (h1 | a_src1)
    R2 = C2 + 1                    # 17  (h2 | a_src2)
    K_eq = meta["K_eq"]
    offs = meta["offs"]
    n_cores = meta["n_cores"]
    TRASH_POS = meta["trash_pos"]

    nc = bacc.Bacc("TRN2", target_bir_lowering=False, debug=False,
                   num_devices=n_cores)

    xT_d = nc.dram_tensor("xT_shard", [F_IN, SH], F32, kind="ExternalInput")
    W1_d = nc.dram_tensor("W1ext", [F_IN, D1 + 2 * H1], F32, kind="ExternalInput")
    W2_d = nc.dram_tensor("W2ext", [D1, C2 + 2], F32, kind="ExternalInput")
    b1_d = nc.dram_tensor("b1v", [1, D1], F32, kind="ExternalInput")
    b2_d = nc.dram_tensor("b2v", [1, C2], F32, kind="ExternalInput")
    grid_d = nc.dram_tensor("grid", [128, GK], I32, kind="ExternalInput")
    id_d = nc.dram_tensor("ident", [128, 128], F32, kind="ExternalInput")
    out_d = nc.dram_tensor("out", [SH, C2], F32, kind="ExternalOutput")

    t1s_d = nc.dram_tensor("t1_shard", [SH, R1], F32)
    t1f_d = nc.dram_tensor("t1_full", [NP, R1], F32, addr_space="Shared")
    ad1_d = nc.dram_tensor("adst1_shard", [SH, H1], F32)
    o1_d = nc.dram_tensor("out1_nat", [SH, D1], F32)
    t2s_d = nc.dram_tensor("t2_shard", [SH, R2], F32)
    t2f_d = nc.dram_tensor("t2_full", [NP, R2], F32, addr_space="Shared")
    ad2_d = nc.dram_tensor("adst2_shard", [SH, 1], F32)

    groups = [list(range(n_cores))]

    with tile.TileContext(nc) as tc:
      for _rep in range(repeat):
          # ---------------- phase A: L1 projection ----------------
          with tc.tile_pool(name="pa", bufs=2) as pa, \
               tc.tile_pool(name="pa1", bufs=1) as pa1, \
               tc.tile_pool(name="psA", bufs=4, space="PSUM") as psA:
              w1_t = pa1.tile([F_IN, D1 + 2 * H1], F32)
              nc.sync.dma_start(out=w1_t[:], in_=W1_d[:])
              xT_t = pa1.tile([F_IN, SH], F32)
              nc.sync.dma_start(out=xT_t[:], in_=xT_d[:])
              st1 = pa1.tile([128, T * R1], F32)
              stA = pa1.tile([128, T * H1], F32)
              for t in range(T):
                  ps = psA.tile([128, D1 + 2 * H1], F32, tag="psA")
                  nc.tensor.matmul(ps[:], lhsT=xT_t[:, t * 128:(t + 1) * 128],
                                   rhs=w1_t[:], start=True, stop=True)
                  nc.vector.tensor_copy(out=st1[:, t * R1:(t + 1) * R1],
                                        in_=ps[:, 0:R1])
                  nc.vector.tensor_copy(out=stA[:, t * H1:(t + 1) * H1],
                                        in_=ps[:, R1:R1 + H1])
              nc.sync.dma_start(
                  out=t1s_d.ap().rearrange("(t p) c -> p t c", p=128),
                  in_=st1[:].rearrange("p (t c) -> p t c", c=R1))
              nc.sync.dma_start(
                  out=ad1_d.ap().rearrange("(t p) c -> p t c", p=128),
                  in_=stA[:].rearrange("p (t c) -> p t c", c=H1))

          nc.gpsimd.collective_compute(
              "AllGather", OP.bypass, replica_groups=groups,
              ins=[t1s_d[:]], outs=[t1f_d[:]])

          with tc.tile_pool(name="poi", bufs=1) as poi:
              pz = poi.tile([1, H1], F32)
              nc.vector.memset(pz[:], POISON)
              nc.sync.dma_start(out=t1f_d[TRASH_POS:TRASH_POS + 1, D1:R1], in_=pz[:])

          # ---------------- phase B: L1 edge aggregation ----------------
          with tc.tile_pool(name="pb", bufs=4) as pb, \
               tc.tile_pool(name="pb1", bufs=1) as pb1:
              grid_t = pb1.tile([128, GK], I32)
              nc.sync.dma_start(out=grid_t[:], in_=grid_d[:])
              ad1_all = pb1.tile([128, T, H1], F32)
              nc.sync.dma_start(out=ad1_all[:],
                                in_=ad1_d.ap().rearrange("(t p) c -> p t c", p=128))
              o1_st = pb1.tile([128, T * D1], F32)
              b1_t = pb1.tile([128, D1], F32)
              nc.sync.dma_start(out=b1_t[:], in_=b1_d[:].to_broadcast([128, D1]))

              for t in range(T):
                  K = K_eq[t]
                  off = offs[t]
                  g = pb.tile([128, K, R1], F32, tag="g1")
                  for j in range(K):
                      nc.gpsimd.indirect_dma_start(
                          out=g[:, j, :], out_offset=None, in_=t1f_d[:],
                          in_offset=bass.IndirectOffsetOnAxis(
                              ap=grid_t[:, off + j:off + j + 1], axis=0))
                  adst = ad1_all[:, t, :]
                  # logits = a_src(slot) + a_dst  -> lrelu -> exp
                  lg = pb.tile([128, K, H1], F32, tag="lg1")
                  nc.vector.tensor_tensor(
                      out=lg[:], in0=g[:, :, D1:R1],
                      in1=adst.unsqueeze(1).to_broadcast([128, K, H1]),
                      op=OP.add)
                  nc.vector.scalar_tensor_tensor(
                      out=lg[:], in0=lg[:], scalar=NEG_SLOPE, in1=lg[:],
                      op0=OP.mult, op1=OP.max)
                  w = pb.tile([128, K, H1], F32, tag="w1")
                  nc.scalar.activation(w[:], lg[:], AF.Exp)
                  # msg = h * w (broadcast over C1 channels), in place on g
                  gh = g[:, :, 0:D1].rearrange("p k (h c) -> p k h c", c=C1)
                  nc.vector.tensor_tensor(
                      out=gh, in0=gh,
                      in1=w[:].unsqueeze(3).to_broadcast([128, K, H1, C1]),
                      op=OP.mult)
                  # reduce slots
                  S = pb.tile([128, D1], F32, tag="S1")
                  nc.vector.tensor_reduce(
                      out=S[:], in_=g[:, :, 0:D1].rearrange("p k c -> p c k"),
                      axis=mybir.AxisListType.X, op=OP.add)
                  z = pb.tile([128, H1], F32, tag="z1")
                  nc.vector.tensor_reduce(
                      out=z[:], in_=w[:].rearrange("p k h -> p h k"),
                      axis=mybir.AxisListType.X, op=OP.add)
                  nc.vector.tensor_scalar_add(z[:], z[:], EPS)
                  rz = pb.tile([128, H1], F32, tag="rz1")
                  nc.vector.reciprocal(rz[:], z[:])
                  o = pb.tile([128, D1], F32, tag="o1")
                  nc.vector.tensor_tensor(
                      out=o[:].rearrange("p (h c) -> p h c", c=C1),
                      in0=S[:].rearrange("p (h c) -> p h c", c=C1),
                      in1=rz[:].unsqueeze(2).to_broadcast([128, H1, C1]),
                      op=OP.mult)
                  # + b1, then ELU
                  nc.vector.tensor_tensor(
                      out=o[:], in0=o[:], in1=b1_t[:], op=OP.add)
                  tmin = pb.tile([128, D1], F32, tag="tm1")
                  nc.vector.tensor_scalar_min(tmin[:], o[:], 0.0)
                  texp = pb.tile([128, D1], F32, tag="te1")
                  nc.scalar.activation(texp[:], tmin[:], AF.Exp)
                  nc.vector.tensor_scalar_max(o[:], o[:], 0.0)
                  nc.vector.scalar_tensor_tensor(
                      out=o1_st[:, t * D1:(t + 1) * D1], in0=texp[:],
                      scalar=-1.0, in1=o[:], op0=OP.add, op1=OP.add)
              nc.sync.dma_start(
                  out=o1_d.ap().rearrange("(t p) c -> p t c", p=128),
                  in_=o1_st[:].rearrange("p (t c) -> p t c", c=D1))

          # ---------------- phase A2: L2 projection ----------------
          with tc.tile_pool(name="pc", bufs=3) as pc, \
               tc.tile_pool(name="pc1", bufs=1) as pc1, \
               tc.tile_pool(name="psC", bufs=4, space="PSUM") as psC:
              id_t = pc1.tile([128, 128], F32)
              nc.sync.dma_start(out=id_t[:], in_=id_d[:])
              w2_t = pc1.tile([D1, C2 + 2], F32)
              nc.sync.dma_start(out=w2_t[:], in_=W2_d[:])
              st2 = pc1.tile([128, T * R2], F32)
              stA2 = pc1.tile([128, T], F32)
              for t in range(T):
                  h = pc.tile([128, D1], F32, tag="h1n")
                  nc.sync.dma_start(out=h[:], in_=o1_d[t * 128:(t + 1) * 128, :])
                  pst = psC.tile([D1, 128], F32, tag="psT")
                  nc.tensor.transpose(pst[:], h[:], id_t[:])
                  hT = pc.tile([D1, 128], F32, tag="hT")
                  nc.vector.tensor_copy(out=hT[:], in_=pst[:])
                  ps2 = psC.tile([128, C2 + 2], F32, tag="ps2")
                  nc.tensor.matmul(ps2[:], lhsT=hT[:], rhs=w2_t[:],
                                   start=True, stop=True)
                  nc.vector.tensor_copy(out=st2[:, t * R2:(t + 1) * R2],
                                        in_=ps2[:, 0:R2])
                  nc.vector.tensor_copy(out=stA2[:, t:t + 1],
                                        in_=ps2[:, R2:R2 + 1])
              nc.sync.dma_start(
                  out=t2s_d.ap().rearrange("(t p) c -> p t c", p=128),
                  in_=st2[:].rearrange("p (t c) -> p t c", c=R2))
              nc.sync.dma_start(
                  out=ad2_d.ap().rearrange("(t p) c -> p t c", p=128),
                  in_=stA2[:].unsqueeze(2))

          nc.gpsimd.collective_compute(
              "AllGather", OP.bypass, replica_groups=groups,
              ins=[t2s_d[:]], outs=[t2f_d[:]])

          with tc.tile_pool(name="poi2", bufs=1) as poi2:
              pz2 = poi2.tile([1, 1], F32)
              nc.vector.memset(pz2[:], POISON)
              nc.sync.dma_start(out=t2f_d[TRASH_POS:TRASH_POS + 1, C2:R2], in_=pz2[:])

          # ---------------- phase C: L2 edge + log_softmax ----------------
          with tc.tile_pool(name="pd", bufs=4) as pd, \
               tc.tile_pool(name="pd1", bufs=1) as pd1:
              grid_t2 = pd1.tile([128, GK], I32)
              nc.sync.dma_start(out=grid_t2[:], in_=grid_d[:])
              ad2_all = pd1.tile([128, T, 1], F32)
              nc.sync.dma_start(out=ad2_all[:],
                                in_=ad2_d.ap().rearrange("(t p) c -> p t c", p=128))
              o2_st = pd1.tile([128, T * C2], F32)
              b2_t = pd1.tile([128, C2], F32)
              nc.sync.dma_start(out=b2_t[:], in_=b2_d[:].to_broadcast([128, C2]))

              for t in range(T):
                  K = K_eq[t]
                  off = offs[t]
                  g2 = pd.tile([128, K, R2], F32, tag="g2")
                  for j in range(K):
                      nc.gpsimd.indirect_dma_start(
                          out=g2[:, j, :], out_offset=None, in_=t2f_d[:],
                          in_offset=bass.IndirectOffsetOnAxis(
                              ap=grid_t2[:, off + j:off + j + 1], axis=0))
                  ad2 = ad2_all[:, t, :]
                  lg2 = pd.tile([128, K], F32, tag="lg2")
                  nc.vector.tensor_tensor(
                      out=lg2[:], in0=g2[:, :, C2],
                      in1=ad2.to_broadcast([128, K]), op=OP.add)
                  nc.vector.scalar_tensor_tensor(
                      out=lg2[:], in0=lg2[:], scalar=NEG_SLOPE, in1=lg2[:],
                      op0=OP.mult, op1=OP.max)
                  w2 = pd.tile([128, K], F32, tag="w2")
                  nc.scalar.activation(w2[:], lg2[:], AF.Exp)
                  nc.vector.tensor_tensor(
                      out=g2[:, :, 0:C2], in0=g2[:, :, 0:C2],
                      in1=w2[:].unsqueeze(2).to_broadcast([128, K, C2]),
                      op=OP.mult)
                  S2 = pd.tile([128, C2], F32, tag="S2")
                  nc.vector.tensor_reduce(
                      out=S2[:], in_=g2[:, :, 0:C2].rearrange("p k c -> p c k"),
                      axis=mybir.AxisListType.X, op=OP.add)
                  z2 = pd.tile([128, 1], F32, tag="z2")
                  nc.vector.tensor_reduce(
                      out=z2[:], in_=w2[:].unsqueeze(1),
                      axis=mybir.AxisListType.X, op=OP.add)
                  nc.vector.tensor_scalar_add(z2[:], z2[:], EPS)
                  rz2 = pd.tile([128, 1], F32, tag="rz2")
                  nc.vector.reciprocal(rz2[:], z2[:])
                  o2 = pd.tile([128, C2], F32, tag="o2")
                  nc.vector.tensor_tensor(
                      out=o2[:], in0=S2[:],
                      in1=rz2[:].to_broadcast([128, C2]), op=OP.mult)
                  nc.vector.tensor_tensor(
                      out=o2[:], in0=o2[:], in1=b2_t[:], op=OP.add)
                  # log_softmax over the 16 classes
                  mx = pd.tile([128, 1], F32, tag="mx")
                  nc.vector.tensor_reduce(out=mx[:], in_=o2[:],
                                          axis=mybir.AxisListType.X, op=OP.max)
                  nc.vector.tensor_scalar(
                      out=o2[:], in0=o2[:], scalar1=mx[:], scalar2=None,
                      op0=OP.subtract)
                  ex = pd.tile([128, C2], F32, tag="ex")
                  nc.scalar.activation(ex[:], o2[:], AF.Exp)
                  sz = pd.tile([128, 1], F32, tag="sz")
                  nc.vector.tensor_reduce(out=sz[:], in_=ex[:],
                                          axis=mybir.AxisListType.X, op=OP.add)
                  lnz = pd.tile([128, 1], F32, tag="lnz")
                  nc.scalar.activation(lnz[:], sz[:], AF.Ln)
                  nc.vector.tensor_scalar(
                      out=o2_st[:, t * C2:(t + 1) * C2], in0=o2[:],
                      scalar1=lnz[:], scalar2=None, op0=OP.subtract)
              nc.sync.dma_start(
                  out=out_d.ap().rearrange("(t p) c -> p t c", p=128),
                  in_=o2_st[:].rearrange("p (t c) -> p t c", c=C2))

    nc.compile()
    return nc


# ------------------------------------------------------------- public entry

_CACHE = {}


def _runner_for(meta):
    key = (meta["NP"], meta["GK"], tuple(meta["K_eq"]), meta["trash_pos"])
    if key in _CACHE:
        return _CACHE[key]
    import jax
    from jax.sharding import Mesh, PartitionSpec
    from jax.experimental.shard_map import shard_map
    from concourse import bass2jax
    from concourse.bass2jax import _bass_exec_p, partition_id_tensor

    nc = build_program(meta)
    bass2jax.install_neuronx_cc_hook()
    partition_name = nc.partition_id_tensor.name if nc.partition_id_tensor else None
    in_names, out_names, out_avals, zero_outs = [], [], [], []
    for alloc in nc.m.functions[0].allocations:
        if not isinstance(alloc, mybir.MemoryLocationSet):
            continue
        name = alloc.memorylocations[0].name
        if alloc.kind == "ExternalInput":
            if name != partition_name:
                in_names.append(name)
        elif alloc.kind == "ExternalOutput":
            out_names.append(name)
            shape = tuple(alloc.tensor_shape)
            dtype = mybir.dt.np(alloc.dtype)
            out_avals.append(jax.core.ShapedArray(shape, dtype))
            zero_outs.append(np.zeros(shape, dtype))
    n_params = len(in_names)
    n_outs = len(out_avals)
    all_in_names = list(in_names) + list(out_names)
    if partition_name is not None:
        all_in_names.append(partition_name)

    def _body(*args):
        operands = list(args)
        if partition_name is not None:
            operands.append(partition_id_tensor())
        outs = _bass_exec_p.bind(
            *operands,
            out_avals=tuple(out_avals),
            in_names=tuple(all_in_names),
            out_names=tuple(out_names),
            lowering_input_output_aliases=(),
            sim_require_finite=True,
            sim_require_nnan=True,
            nc=nc,
        )
        return tuple(outs)

    n_cores = meta["n_cores"]
    devices = jax.devices()[:n_cores]
    mesh = Mesh(np.asarray(devices), ("core",))
    in_specs = (PartitionSpec("core"),) * (n_params + n_outs)
    out_specs = (PartitionSpec("core"),) * n_outs
    sharded = jax.jit(
        shard_map(_body, mesh=mesh, in_specs=in_specs, out_specs=out_specs,
                  check_rep=False),
        keep_unused=True,
    )

    def run(in_maps):
        import jax
        concat_in = [
            np.concatenate([np.asarray(in_maps[c][nm]) for c in range(n_cores)], 0)
            for nm in in_names
        ] + [np.concatenate([z] * n_cores, 0) for z in zero_outs]
        staged = [jax.device_put(a) for a in concat_in]
        outs = sharded(*staged)
        jax.block_until_ready(outs)
        outs_np = [np.asarray(o) for o in outs]
        results = []
        for c in range(n_cores):
            m = {}
            for i, nm in enumerate(out_names):
                sh = out_avals[i].shape
                m[nm] = outs_np[i][c * sh[0]:(c + 1) * sh[0]]
            results.append(m)
        return results

    _CACHE[key] = run
    return run


def kernel(x, edge_index, W1, att_src1, att_dst1, b1, W2, att_src2,
           att_dst2, b2):
    """Full-input GAT forward on 8 NeuronCores; returns [N, C2] float32."""
    x = np.asarray(x)
    N = x.shape[0]
    in_maps, meta, perms = host_prep(x, edge_index, W1, att_src1, att_dst1,
                                     b1, W2, att_src2, att_dst2, b2, n_cores=8)
    run = _runner_for(meta)
    results = run(in_maps)
    SH = meta["SH"]
    C2 = meta["C2"]
    nat = np.empty((meta["NP"], C2), np.float32)
    for c in range(meta["n_cores"]):
        nat[c * SH + perms[c]] = results[c]["out"]
    return np.ascontiguousarray(nat[:N], dtype=np.float32)



# revision 5
# speedup vs baseline: 1.3532x; 1.3532x over previous
"""GAT kernel builder for TRN2 (8-core SPMD, dst-sharded ELL layout).

Design:
- Nodes padded to NP = 8*SH; core c owns dst/node rows [c*SH, (c+1)*SH).
- Per core, dsts are degree-sorted (ascending, pads first); edges stored in
  an ELL slot grid per 128-dst tile: grid[:, off_t + j] = src id of slot j
  (pad slots -> TRASH node NP-1, whose a_src is poisoned to -1e30 so its
  exp(lrelu(...)) underflows to exactly 0).
- K_t (slots per tile) equalized across cores (SPMD: one program).
- Gather: one indirect_dma_start per slot column (128 rows, one per
  partition = dst). f32 end-to-end.
- Layer tables: T1 [NP,72] = [h1(64) | a_src1(8)]; a_dst1 kept per-shard.
  T2 [NP,17] = [h2(16) | a_src2(1)]; a_dst2 per-shard. AllGather between
  layers via collective.
"""
import numpy as np

import concourse.bacc as bacc
import concourse.bass as bass
import concourse.mybir as mybir
import concourse.tile as tile

F32 = mybir.dt.float32
I32 = mybir.dt.int32
AF = mybir.ActivationFunctionType
OP = mybir.AluOpType

NEG_SLOPE = 0.2
EPS = 1e-16
POISON = -1.0e30


# ---------------------------------------------------------------- host prep

def host_prep(x, edge_index, W1, att_src1, att_dst1, b1, W2, att_src2,
              att_dst2, b2, n_cores=8):
    """Pure index/layout prep on host. Returns (in_maps, meta)."""
    N = x.shape[0]
    F_IN = x.shape[1]
    H1, C1 = att_src1.shape
    C2 = att_src2.shape[1]
    SH = -(-N // (128 * n_cores)) * 128          # shard rows, mult of 128
    NP = SH * n_cores
    T = SH // 128                                 # dst tiles per core
    TRASH = NP - 1

    src = np.concatenate([np.asarray(edge_index[0]), np.arange(N)]).astype(np.int64)
    dst = np.concatenate([np.asarray(edge_index[1]), np.arange(N)]).astype(np.int64)

    # per-core CSR by dst
    core_of = dst // SH
    perms = []            # [n_cores][128, T] int32  natural local row of sorted pos
    grids_per_core = []   # [n_cores][T] list of [128, K_t] arrays (pre-equalize)
    Ks = np.zeros((n_cores, T), dtype=np.int64)
    deg_sorted_idx = []
    for c in range(n_cores):
        m = core_of == c
        s_c = src[m]
        d_loc = (dst[m] - c * SH).astype(np.int64)
        deg = np.bincount(d_loc, minlength=SH)
        order = np.argsort(deg, kind="stable")    # ascending; zero-degree pads first
        deg_sorted_idx.append(order)
        # CSR over local dst
        sort_by_d = np.argsort(d_loc, kind="stable")
        s_sorted = s_c[sort_by_d]
        rowptr = np.zeros(SH + 1, dtype=np.int64)
        np.cumsum(deg, out=rowptr[1:])
        perms.append(order)
        tiles = []
        for t in range(T):
            dts = order[t * 128:(t + 1) * 128]
            K_t = max(int(deg[dts].max()), 1)
            Ks[c, t] = K_t
            g = np.full((128, K_t), TRASH, dtype=np.int32)
            for p, dl in enumerate(dts):
                a, b = rowptr[dl], rowptr[dl + 1]
                g[p, : b - a] = s_sorted[a:b]
            tiles.append(g)
        grids_per_core.append(tiles)

    # equalize K_t across cores
    K_eq = Ks.max(axis=0).astype(np.int64)        # [T]
    offs = np.zeros(T + 1, dtype=np.int64)
    np.cumsum(K_eq, out=offs[1:])
    GK = int(offs[-1])
    grids = []
    for c in range(n_cores):
        g_all = np.full((128, GK), TRASH, dtype=np.int32)
        for t in range(T):
            g = grids_per_core[c][t]
            g_all[:, offs[t]:offs[t] + g.shape[1]] = g
        grids.append(g_all)

    # global position map: node n -> its AllGather row (cores emit sorted shards)
    posmap = np.zeros(NP, dtype=np.int32)
    for c in range(n_cores):
        inv = np.empty(SH, dtype=np.int64)
        inv[perms[c]] = np.arange(SH)
        posmap[c * SH:(c + 1) * SH] = (c * SH + inv).astype(np.int32)
    grids = [posmap[g] for g in grids]
    trash_pos = int(posmap[NP - 1])

    # x transposed + padded; per-core columns in SORTED order
    xT = np.zeros((F_IN, NP), dtype=np.float32)
    xT[:, :N] = np.asarray(x, dtype=np.float32).T

    # weight prep (block-diag fold of attention vectors into the projection)
    W1 = np.asarray(W1, np.float32)
    W2 = np.asarray(W2, np.float32)
    BDs = np.zeros((H1 * C1, H1), np.float32)
    BDd = np.zeros((H1 * C1, H1), np.float32)
    for h in range(H1):
        BDs[h * C1:(h + 1) * C1, h] = np.asarray(att_src1, np.float32)[h]
        BDd[h * C1:(h + 1) * C1, h] = np.asarray(att_dst1, np.float32)[h]
    W1ext = np.concatenate([W1, W1 @ BDs, W1 @ BDd], axis=1)          # [F_IN, 80]
    W2ext = np.concatenate(
        [W2, W2 @ np.asarray(att_src2, np.float32).reshape(-1, 1),
         W2 @ np.asarray(att_dst2, np.float32).reshape(-1, 1)], axis=1)  # [64, 18]

    ident = np.eye(128, dtype=np.float32)

    in_maps = []
    for c in range(n_cores):
        in_maps.append({
            "xT_shard": np.ascontiguousarray(xT[:, c * SH + perms[c]]),
            "W1ext": W1ext,
            "W2ext": W2ext,
            "b1v": np.asarray(b1, np.float32).reshape(1, -1),
            "b2v": np.asarray(b2, np.float32).reshape(1, -1),
            "grid": grids[c],
            "ident": ident,
        })
    meta = dict(N=N, NP=NP, SH=SH, T=T, GK=GK, K_eq=K_eq.tolist(),
                offs=offs.tolist(), F_IN=F_IN, H1=H1, C1=C1, C2=C2,
                n_cores=n_cores, trash_pos=trash_pos)
    return in_maps, meta, perms


# ------------------------------------------------------------- device build

def build_program(meta, repeat=1):
    NP, SH, T, GK = meta["NP"], meta["SH"], meta["T"], meta["GK"]
    F_IN = meta["F_IN"]
    H1, C1, C2 = meta["H1"], meta["C1"], meta["C2"]
    D1 = H1 * C1                   # 64
    R1 = D1 + H1                   # 72  (h1 | a_src1)
    R2 = C2 + 1                    # 17  (h2 | a_src2)
    K_eq = meta["K_eq"]
    offs = meta["offs"]
    n_cores = meta["n_cores"]
    TRASH_POS = meta["trash_pos"]

    nc = bacc.Bacc("TRN2", target_bir_lowering=False, debug=False,
                   num_devices=n_cores)

    xT_d = nc.dram_tensor("xT_shard", [F_IN, SH], F32, kind="ExternalInput")
    W1_d = nc.dram_tensor("W1ext", [F_IN, D1 + 2 * H1], F32, kind="ExternalInput")
    W2_d = nc.dram_tensor("W2ext", [D1, C2 + 2], F32, kind="ExternalInput")
    b1_d = nc.dram_tensor("b1v", [1, D1], F32, kind="ExternalInput")
    b2_d = nc.dram_tensor("b2v", [1, C2], F32, kind="ExternalInput")
    grid_d = nc.dram_tensor("grid", [128, GK], I32, kind="ExternalInput")
    id_d = nc.dram_tensor("ident", [128, 128], F32, kind="ExternalInput")
    out_d = nc.dram_tensor("out", [SH, C2], F32, kind="ExternalOutput")

    t1s_d = nc.dram_tensor("t1_shard", [SH, R1], F32)
    t1f_d = nc.dram_tensor("t1_full", [NP, R1], F32, addr_space="Shared")
    ad1_d = nc.dram_tensor("adst1_shard", [SH, H1], F32)
    o1_d = nc.dram_tensor("out1_nat", [SH, D1], F32)
    t2s_d = nc.dram_tensor("t2_shard", [SH, R2], F32)
    t2f_d = nc.dram_tensor("t2_full", [NP, R2], F32, addr_space="Shared")
    ad2_d = nc.dram_tensor("adst2_shard", [SH, 1], F32)

    groups = [list(range(n_cores))]

    with tile.TileContext(nc) as tc:
      for _rep in range(repeat):
          # ---------------- phase A: L1 projection ----------------
          with tc.tile_pool(name="pa", bufs=2) as pa, \
               tc.tile_pool(name="pa1", bufs=1) as pa1, \
               tc.tile_pool(name="psA", bufs=4, space="PSUM") as psA:
              w1_t = pa1.tile([F_IN, D1 + 2 * H1], F32)
              nc.sync.dma_start(out=w1_t[:], in_=W1_d[:])
              xT_t = pa1.tile([F_IN, SH], F32)
              nc.sync.dma_start(out=xT_t[:], in_=xT_d[:])
              st1 = pa1.tile([128, T * R1], F32)
              stA = pa1.tile([128, T * H1], F32)
              for t in range(T):
                  ps = psA.tile([128, D1 + 2 * H1], F32, tag="psA")
                  nc.tensor.matmul(ps[:], lhsT=xT_t[:, t * 128:(t + 1) * 128],
                                   rhs=w1_t[:], start=True, stop=True)
                  nc.vector.tensor_copy(out=st1[:, t * R1:(t + 1) * R1],
                                        in_=ps[:, 0:R1])
                  nc.vector.tensor_copy(out=stA[:, t * H1:(t + 1) * H1],
                                        in_=ps[:, R1:R1 + H1])
              nc.sync.dma_start(
                  out=t1s_d.ap().rearrange("(t p) c -> p t c", p=128),
                  in_=st1[:].rearrange("p (t c) -> p t c", c=R1))
              nc.sync.dma_start(
                  out=ad1_d.ap().rearrange("(t p) c -> p t c", p=128),
                  in_=stA[:].rearrange("p (t c) -> p t c", c=H1))

          nc.gpsimd.collective_compute(
              "AllGather", OP.bypass, replica_groups=groups,
              ins=[t1s_d[:]], outs=[t1f_d[:]])

          with tc.tile_pool(name="poi", bufs=1) as poi:
              pz = poi.tile([1, H1], F32)
              nc.vector.memset(pz[:], POISON)
              nc.sync.dma_start(out=t1f_d[TRASH_POS:TRASH_POS + 1, D1:R1], in_=pz[:])

          # ---------------- phase B: L1 edge aggregation ----------------
          with tc.tile_pool(name="pb", bufs=4) as pb, \
               tc.tile_pool(name="pb1", bufs=1) as pb1:
              grid_t = pb1.tile([128, GK], I32)
              nc.sync.dma_start(out=grid_t[:], in_=grid_d[:])
              ad1_all = pb1.tile([128, T, H1], F32)
              nc.sync.dma_start(out=ad1_all[:],
                                in_=ad1_d.ap().rearrange("(t p) c -> p t c", p=128))
              o1_st = pb1.tile([128, T * D1], F32)
              b1_t = pb1.tile([128, D1], F32)
              nc.sync.dma_start(out=b1_t[:], in_=b1_d[:].to_broadcast([128, D1]))

              for t in range(T):
                  K = K_eq[t]
                  off = offs[t]
                  g = pb.tile([128, K, R1], F32, tag="g1")
                  nc.gpsimd.indirect_dma_start(
                      out=g[:], out_offset=None, in_=t1f_d[:],
                      in_offset=bass.IndirectOffsetOnAxis(
                          ap=grid_t[:, off:off + K], axis=0))
                  adst = ad1_all[:, t, :]
                  # logits = a_src(slot) + a_dst  -> lrelu -> exp
                  lg = pb.tile([128, K, H1], F32, tag="lg1")
                  nc.vector.tensor_tensor(
                      out=lg[:], in0=g[:, :, D1:R1],
                      in1=adst.unsqueeze(1).to_broadcast([128, K, H1]),
                      op=OP.add)
                  nc.vector.scalar_tensor_tensor(
                      out=lg[:], in0=lg[:], scalar=NEG_SLOPE, in1=lg[:],
                      op0=OP.mult, op1=OP.max)
                  w = pb.tile([128, K, H1], F32, tag="w1")
                  nc.scalar.activation(w[:], lg[:], AF.Exp)
                  # msg = h * w (broadcast over C1 channels), in place on g
                  gh = g[:, :, 0:D1].rearrange("p k (h c) -> p k h c", c=C1)
                  nc.vector.tensor_tensor(
                      out=gh, in0=gh,
                      in1=w[:].unsqueeze(3).to_broadcast([128, K, H1, C1]),
                      op=OP.mult)
                  # reduce slots
                  S = pb.tile([128, D1], F32, tag="S1")
                  nc.vector.tensor_reduce(
                      out=S[:], in_=g[:, :, 0:D1].rearrange("p k c -> p c k"),
                      axis=mybir.AxisListType.X, op=OP.add)
                  z = pb.tile([128, H1], F32, tag="z1")
                  nc.vector.tensor_reduce(
                      out=z[:], in_=w[:].rearrange("p k h -> p h k"),
                      axis=mybir.AxisListType.X, op=OP.add)
                  nc.vector.tensor_scalar_add(z[:], z[:], EPS)
                  rz = pb.tile([128, H1], F32, tag="rz1")
                  nc.vector.reciprocal(rz[:], z[:])
                  o = pb.tile([128, D1], F32, tag="o1")
                  nc.vector.tensor_tensor(
                      out=o[:].rearrange("p (h c) -> p h c", c=C1),
                      in0=S[:].rearrange("p (h c) -> p h c", c=C1),
                      in1=rz[:].unsqueeze(2).to_broadcast([128, H1, C1]),
                      op=OP.mult)
                  # + b1, then ELU
                  nc.vector.tensor_tensor(
                      out=o[:], in0=o[:], in1=b1_t[:], op=OP.add)
                  tmin = pb.tile([128, D1], F32, tag="tm1")
                  nc.vector.tensor_scalar_min(tmin[:], o[:], 0.0)
                  texp = pb.tile([128, D1], F32, tag="te1")
                  nc.scalar.activation(texp[:], tmin[:], AF.Exp)
                  nc.vector.tensor_scalar_max(o[:], o[:], 0.0)
                  nc.vector.scalar_tensor_tensor(
                      out=o1_st[:, t * D1:(t + 1) * D1], in0=texp[:],
                      scalar=-1.0, in1=o[:], op0=OP.add, op1=OP.add)
              nc.sync.dma_start(
                  out=o1_d.ap().rearrange("(t p) c -> p t c", p=128),
                  in_=o1_st[:].rearrange("p (t c) -> p t c", c=D1))

          # ---------------- phase A2: L2 projection ----------------
          with tc.tile_pool(name="pc", bufs=3) as pc, \
               tc.tile_pool(name="pc1", bufs=1) as pc1, \
               tc.tile_pool(name="psC", bufs=4, space="PSUM") as psC:
              id_t = pc1.tile([128, 128], F32)
              nc.sync.dma_start(out=id_t[:], in_=id_d[:])
              w2_t = pc1.tile([D1, C2 + 2], F32)
              nc.sync.dma_start(out=w2_t[:], in_=W2_d[:])
              st2 = pc1.tile([128, T * R2], F32)
              stA2 = pc1.tile([128, T], F32)
              for t in range(T):
                  h = pc.tile([128, D1], F32, tag="h1n")
                  nc.sync.dma_start(out=h[:], in_=o1_d[t * 128:(t + 1) * 128, :])
                  pst = psC.tile([D1, 128], F32, tag="psT")
                  nc.tensor.transpose(pst[:], h[:], id_t[:])
                  hT = pc.tile([D1, 128], F32, tag="hT")
                  nc.vector.tensor_copy(out=hT[:], in_=pst[:])
                  ps2 = psC.tile([128, C2 + 2], F32, tag="ps2")
                  nc.tensor.matmul(ps2[:], lhsT=hT[:], rhs=w2_t[:],
                                   start=True, stop=True)
                  nc.vector.tensor_copy(out=st2[:, t * R2:(t + 1) * R2],
                                        in_=ps2[:, 0:R2])
                  nc.vector.tensor_copy(out=stA2[:, t:t + 1],
                                        in_=ps2[:, R2:R2 + 1])
              nc.sync.dma_start(
                  out=t2s_d.ap().rearrange("(t p) c -> p t c", p=128),
                  in_=st2[:].rearrange("p (t c) -> p t c", c=R2))
              nc.sync.dma_start(
                  out=ad2_d.ap().rearrange("(t p) c -> p t c", p=128),
                  in_=stA2[:].unsqueeze(2))

          nc.gpsimd.collective_compute(
              "AllGather", OP.bypass, replica_groups=groups,
              ins=[t2s_d[:]], outs=[t2f_d[:]])

          with tc.tile_pool(name="poi2", bufs=1) as poi2:
              pz2 = poi2.tile([1, 1], F32)
              nc.vector.memset(pz2[:], POISON)
              nc.sync.dma_start(out=t2f_d[TRASH_POS:TRASH_POS + 1, C2:R2], in_=pz2[:])

          # ---------------- phase C: L2 edge + log_softmax ----------------
          with tc.tile_pool(name="pd", bufs=4) as pd, \
               tc.tile_pool(name="pd1", bufs=1) as pd1:
              grid_t2 = pd1.tile([128, GK], I32)
              nc.sync.dma_start(out=grid_t2[:], in_=grid_d[:])
              ad2_all = pd1.tile([128, T, 1], F32)
              nc.sync.dma_start(out=ad2_all[:],
                                in_=ad2_d.ap().rearrange("(t p) c -> p t c", p=128))
              o2_st = pd1.tile([128, T * C2], F32)
              b2_t = pd1.tile([128, C2], F32)
              nc.sync.dma_start(out=b2_t[:], in_=b2_d[:].to_broadcast([128, C2]))

              for t in range(T):
                  K = K_eq[t]
                  off = offs[t]
                  g2 = pd.tile([128, K, R2], F32, tag="g2")
                  nc.gpsimd.indirect_dma_start(
                      out=g2[:], out_offset=None, in_=t2f_d[:],
                      in_offset=bass.IndirectOffsetOnAxis(
                          ap=grid_t2[:, off:off + K], axis=0))
                  ad2 = ad2_all[:, t, :]
                  lg2 = pd.tile([128, K], F32, tag="lg2")
                  nc.vector.tensor_tensor(
                      out=lg2[:], in0=g2[:, :, C2],
                      in1=ad2.to_broadcast([128, K]), op=OP.add)
                  nc.vector.scalar_tensor_tensor(
                      out=lg2[:], in0=lg2[:], scalar=NEG_SLOPE, in1=lg2[:],
                      op0=OP.mult, op1=OP.max)
                  w2 = pd.tile([128, K], F32, tag="w2")
                  nc.scalar.activation(w2[:], lg2[:], AF.Exp)
                  nc.vector.tensor_tensor(
                      out=g2[:, :, 0:C2], in0=g2[:, :, 0:C2],
                      in1=w2[:].unsqueeze(2).to_broadcast([128, K, C2]),
                      op=OP.mult)
                  S2 = pd.tile([128, C2], F32, tag="S2")
                  nc.vector.tensor_reduce(
                      out=S2[:], in_=g2[:, :, 0:C2].rearrange("p k c -> p c k"),
                      axis=mybir.AxisListType.X, op=OP.add)
                  z2 = pd.tile([128, 1], F32, tag="z2")
                  nc.vector.tensor_reduce(
                      out=z2[:], in_=w2[:].unsqueeze(1),
                      axis=mybir.AxisListType.X, op=OP.add)
                  nc.vector.tensor_scalar_add(z2[:], z2[:], EPS)
                  rz2 = pd.tile([128, 1], F32, tag="rz2")
                  nc.vector.reciprocal(rz2[:], z2[:])
                  o2 = pd.tile([128, C2], F32, tag="o2")
                  nc.vector.tensor_tensor(
                      out=o2[:], in0=S2[:],
                      in1=rz2[:].to_broadcast([128, C2]), op=OP.mult)
                  nc.vector.tensor_tensor(
                      out=o2[:], in0=o2[:], in1=b2_t[:], op=OP.add)
                  # log_softmax over the 16 classes
                  mx = pd.tile([128, 1], F32, tag="mx")
                  nc.vector.tensor_reduce(out=mx[:], in_=o2[:],
                                          axis=mybir.AxisListType.X, op=OP.max)
                  nc.vector.tensor_scalar(
                      out=o2[:], in0=o2[:], scalar1=mx[:], scalar2=None,
                      op0=OP.subtract)
                  ex = pd.tile([128, C2], F32, tag="ex")
                  nc.scalar.activation(ex[:], o2[:], AF.Exp)
                  sz = pd.tile([128, 1], F32, tag="sz")
                  nc.vector.tensor_reduce(out=sz[:], in_=ex[:],
                                          axis=mybir.AxisListType.X, op=OP.add)
                  lnz = pd.tile([128, 1], F32, tag="lnz")
                  nc.scalar.activation(lnz[:], sz[:], AF.Ln)
                  nc.vector.tensor_scalar(
                      out=o2_st[:, t * C2:(t + 1) * C2], in0=o2[:],
                      scalar1=lnz[:], scalar2=None, op0=OP.subtract)
              nc.sync.dma_start(
                  out=out_d.ap().rearrange("(t p) c -> p t c", p=128),
                  in_=o2_st[:].rearrange("p (t c) -> p t c", c=C2))

    nc.compile()
    return nc


# ------------------------------------------------------------- public entry

_CACHE = {}


def _runner_for(meta):
    key = (meta["NP"], meta["GK"], tuple(meta["K_eq"]), meta["trash_pos"])
    if key in _CACHE:
        return _CACHE[key]
    import jax
    from jax.sharding import Mesh, PartitionSpec
    from jax.experimental.shard_map import shard_map
    from concourse import bass2jax
    from concourse.bass2jax import _bass_exec_p, partition_id_tensor

    nc = build_program(meta)
    bass2jax.install_neuronx_cc_hook()
    partition_name = nc.partition_id_tensor.name if nc.partition_id_tensor else None
    in_names, out_names, out_avals, zero_outs = [], [], [], []
    for alloc in nc.m.functions[0].allocations:
        if not isinstance(alloc, mybir.MemoryLocationSet):
            continue
        name = alloc.memorylocations[0].name
        if alloc.kind == "ExternalInput":
            if name != partition_name:
                in_names.append(name)
        elif alloc.kind == "ExternalOutput":
            out_names.append(name)
            shape = tuple(alloc.tensor_shape)
            dtype = mybir.dt.np(alloc.dtype)
            out_avals.append(jax.core.ShapedArray(shape, dtype))
            zero_outs.append(np.zeros(shape, dtype))
    n_params = len(in_names)
    n_outs = len(out_avals)
    all_in_names = list(in_names) + list(out_names)
    if partition_name is not None:
        all_in_names.append(partition_name)

    def _body(*args):
        operands = list(args)
        if partition_name is not None:
            operands.append(partition_id_tensor())
        outs = _bass_exec_p.bind(
            *operands,
            out_avals=tuple(out_avals),
            in_names=tuple(all_in_names),
            out_names=tuple(out_names),
            lowering_input_output_aliases=(),
            sim_require_finite=True,
            sim_require_nnan=True,
            nc=nc,
        )
        return tuple(outs)

    n_cores = meta["n_cores"]
    devices = jax.devices()[:n_cores]
    mesh = Mesh(np.asarray(devices), ("core",))
    in_specs = (PartitionSpec("core"),) * (n_params + n_outs)
    out_specs = (PartitionSpec("core"),) * n_outs
    sharded = jax.jit(
        shard_map(_body, mesh=mesh, in_specs=in_specs, out_specs=out_specs,
                  check_rep=False),
        keep_unused=True,
    )

    def run(in_maps):
        import jax
        concat_in = [
            np.concatenate([np.asarray(in_maps[c][nm]) for c in range(n_cores)], 0)
            for nm in in_names
        ] + [np.concatenate([z] * n_cores, 0) for z in zero_outs]
        staged = [jax.device_put(a) for a in concat_in]
        outs = sharded(*staged)
        jax.block_until_ready(outs)
        outs_np = [np.asarray(o) for o in outs]
        results = []
        for c in range(n_cores):
            m = {}
            for i, nm in enumerate(out_names):
                sh = out_avals[i].shape
                m[nm] = outs_np[i][c * sh[0]:(c + 1) * sh[0]]
            results.append(m)
        return results

    _CACHE[key] = run
    return run


def kernel(x, edge_index, W1, att_src1, att_dst1, b1, W2, att_src2,
           att_dst2, b2):
    """Full-input GAT forward on 8 NeuronCores; returns [N, C2] float32."""
    x = np.asarray(x)
    N = x.shape[0]
    in_maps, meta, perms = host_prep(x, edge_index, W1, att_src1, att_dst1,
                                     b1, W2, att_src2, att_dst2, b2, n_cores=8)
    run = _runner_for(meta)
    results = run(in_maps)
    SH = meta["SH"]
    C2 = meta["C2"]
    nat = np.empty((meta["NP"], C2), np.float32)
    for c in range(meta["n_cores"]):
        nat[c * SH + perms[c]] = results[c]["out"]
    return np.ascontiguousarray(nat[:N], dtype=np.float32)



# revision 6
# speedup vs baseline: 1.6682x; 1.2328x over previous
"""GAT kernel for TRN2, v2: dma_gather (SWDGE Ant ucode) edge gathers.

Design (vs v1's per-column indirect1d):
- Gather primitive: gpsimd.dma_gather, bf16 256B table rows, 16 idx/desc,
  ~2.8ns/idx at 8 cores (vs ~11ns/edge for indirect1d).
- int16 idx limit (<32768 rows) handled by splitting the node table into 4
  contiguous chunks = core pairs {0,1},{2,3},{4,5},{6,7} (2*SH<=32768 rows).
- Per-dst chunk balance: nodes are assigned to cores by a batched greedy
  that balances each dst's in-edges across the 4 chunks, so per-(tile,chunk)
  ELL widths ~= ceil(K/4) and idx inflation stays ~15-25%.
- Landing layout (transpose=False): idx i -> (partition i%128, col i//128),
  i.e. column-major slot grid per 128-dst tile — same compute structure as
  v1 (logits from cols 64:72, messages cols 0:64, per-partition reduces).
- Tables: T1 [NP,128]bf16 = [h1(64)|a_src1(8)|pad]; T2 [NP,128]bf16 =
  [h2(16)|a_src2(1)|pad]. a_dst kept per-shard f32. AllGather between
  layers. Chunk-base rows are pad nodes; their a_src is poisoned to -1e30
  after each AllGather so pad slots contribute exactly 0.
"""
import numpy as np

import concourse.bacc as bacc
import concourse.bass as bass
import concourse.mybir as mybir
import concourse.tile as tile
from concourse import library_config

F32 = mybir.dt.float32
BF16 = mybir.dt.bfloat16
I16 = mybir.dt.int16
AF = mybir.ActivationFunctionType
OP = mybir.AluOpType

NEG_SLOPE = 0.2
EPS = 1e-16
POISON = -1.0e30
MAXW = 8                      # max ELL columns per dma_gather (NI<=1024)


# ---------------------------------------------------------------- host prep

def _ranges(counts):
    """[0..c0), [0..c1), ... concatenated."""
    total = int(counts.sum())
    out = np.arange(total, dtype=np.int64)
    starts = np.zeros(len(counts), np.int64)
    np.cumsum(counts[:-1], out=starts[1:])
    return out - np.repeat(starts, counts)


def _assign_classes(src, dst, N, SH, rng):
    """Greedy balanced 4-coloring of nodes: per dst, spread its in-edge srcs
    evenly over the 4 classes. Returns cls[N] in 0..3."""
    deg = np.bincount(dst, minlength=N)
    cap = -(-deg // 4)
    cnt = np.zeros((N, 4), np.int32)
    order_e = np.argsort(src, kind="stable")
    s_sorted = src[order_e]
    d_sorted = dst[order_e]
    rowptr = np.zeros(N + 1, np.int64)
    np.cumsum(np.bincount(s_sorted, minlength=N), out=rowptr[1:])
    n_pad = 8 * SH - N
    classcap = 2 * SH - 2 * (n_pad // 8)   # real-node capacity per class
    fill = np.zeros(4, np.int64)
    cls = np.full(N, -1, np.int8)
    perm = rng.permutation(N)
    B = 4096
    for i in range(0, N, B):
        batch = perm[i:i + B]
        counts = (rowptr[batch + 1] - rowptr[batch]).astype(np.int64)
        flat = np.repeat(rowptr[batch], counts) + _ranges(counts)
        dd = d_sorted[flat]
        owner = np.repeat(np.arange(len(batch)), counts)
        over = cnt[dd].astype(np.float64)
        over += 1e3 * (cnt[dd] >= cap[dd][:, None])             # [Eb, 4]
        sc = np.zeros((len(batch), 4), np.float64)
        np.add.at(sc, owner, over)
        sc += rng.random(sc.shape) * 0.01
        full_cls = fill >= classcap
        sc[:, full_cls] = np.inf
        c_star = np.argmin(sc, axis=1)
        for _ in range(3):        # quota repair
            done = True
            for c in range(4):
                sel = np.where(c_star == c)[0]
                overflow = int(fill[c] + len(sel) - classcap)
                if overflow > 0:
                    done = False
                    move = sel[np.argsort(sc[sel, c])[::-1][:overflow]]
                    sc[move, c] = np.inf
                    c_star[move] = np.argmin(sc[move], axis=1)
            if done:
                break
        cls[batch] = c_star.astype(np.int8)
        fill += np.bincount(c_star, minlength=4)
        np.add.at(cnt, (dd, c_star[owner]), 1)
    return cls


def host_prep(x, edge_index, W1, att_src1, att_dst1, b1, W2, att_src2,
              att_dst2, b2, n_cores=8):
    N = x.shape[0]
    F_IN = x.shape[1]
    H1, C1 = att_src1.shape
    C2 = att_src2.shape[1]
    SH = -(-N // (128 * n_cores)) * 128
    NP = SH * n_cores
    T = SH // 128
    CH = 2 * SH                       # chunk rows
    rng = np.random.default_rng(12345)

    src = np.concatenate([np.asarray(edge_index[0]), np.arange(N)]).astype(np.int64)
    dst = np.concatenate([np.asarray(edge_index[1]), np.arange(N)]).astype(np.int64)

    # ---- balanced class assignment; class c -> cores {2c, 2c+1}
    cls = _assign_classes(src, dst, N, SH, rng)
    n_pad = NP - N
    pads_per_core = n_pad // n_cores
    real_cap = SH - pads_per_core
    core_of_node = np.full(NP, -1, np.int64)
    # nodes of class c fill core 2c then 2c+1
    for c in range(4):
        nodes = np.where(cls == c)[0]
        assert len(nodes) <= 2 * real_cap, (c, len(nodes), real_cap)
        core_of_node[nodes[:real_cap]] = 2 * c
        core_of_node[nodes[real_cap:]] = 2 * c + 1
    # pads fill remaining capacity
    pad_ids = np.arange(N, NP)
    counts = np.bincount(core_of_node[:N][core_of_node[:N] >= 0],
                         minlength=n_cores)
    pp = 0
    for cidx in range(n_cores):
        k = SH - int(counts[cidx])
        core_of_node[pad_ids[pp:pp + k]] = cidx
        pp += k
    assert pp == n_pad

    # ---- per-core degree sort (pads/deg0 first), posmap
    deg_all = np.bincount(dst, minlength=NP)
    perms = []          # natural node ids (local index) in sorted order
    node_lists = []
    posmap = np.zeros(NP, dtype=np.int64)
    for cidx in range(n_cores):
        nodes = np.where(core_of_node == cidx)[0]
        order = np.argsort(deg_all[nodes], kind="stable")
        snodes = nodes[order]
        node_lists.append(snodes)
        posmap[snodes] = cidx * SH + np.arange(SH)
    # each chunk's base row must be a zero-degree pad (poison target)
    for c in range(4):
        base_node = node_lists[2 * c][0]
        assert deg_all[base_node] == 0 and base_node >= N, "chunk base not pad"

    # ---- per-core ELL grids split by class
    src_pos = posmap[src]
    dst_core = core_of_node[dst]
    sched = None                       # [(t, [(cls, wlen, ioff, goff)...])]
    Wtc = np.zeros((n_cores, T, 4), np.int64)
    grids = []                         # per core: [T][4] -> [128, w] local idx
    for cidx in range(n_cores):
        m = dst_core == cidx
        s_p = src_pos[m]
        d_loc = posmap[dst[m]] - cidx * SH
        order = np.lexsort((s_p // CH, d_loc))     # by dst, then class
        s_p = s_p[order]
        d_loc = d_loc[order]
        e_cls = (s_p // CH).astype(np.int64)
        # CSR boundaries per (dst, class)
        key = d_loc * 4 + e_cls
        cnts = np.bincount(key, minlength=SH * 4).reshape(SH, 4)
        Wtc[cidx] = cnts.reshape(T, 128, 4).max(axis=1)
        grids.append((s_p, d_loc, e_cls, cnts))

    W_eq = Wtc.max(axis=0)             # [T, 4] equalized widths
    # schedule: per tile, windows of <= MAXW columns
    sched = []
    ioff = 0
    for t in range(T):
        ent = []
        goff = 0
        for c in range(4):
            w = int(W_eq[t, c])
            if t == 0 and c == 0 and W_eq.sum() == 0:
                w = 1
            while w > 0:
                wl = min(w, MAXW)
                # balance the split (9 -> 5+4 not 8+1)
                nwin = -(-w // MAXW)
                wl = -(-w // nwin)
                ent.append((c, wl, ioff, goff))
                ioff += wl * 8        # idx cols (128*wl/16)
                goff += wl
                w -= wl
        if goff == 0:                  # fully empty tile: one pad window
            ent.append((0, 1, ioff, 0))
            ioff += 8
            goff = 1
        sched.append((ent, goff))
    TOTW = ioff
    K2 = [s[1] for s in sched]

    # ---- idx tensors per core
    idx_all = []
    for cidx in range(n_cores):
        s_p, d_loc, e_cls, cnts = grids[cidx]
        # rowptr over (dst, class)
        key_counts = cnts.reshape(-1)
        rp = np.zeros(SH * 4 + 1, np.int64)
        np.cumsum(key_counts, out=rp[1:])
        arr = np.zeros((128, TOTW), np.int16)
        for t in range(T):
            ent, _ = sched[t]
            consumed = {}
            for (c, wl, io, go) in ent:
                Lbuf = np.zeros((wl * 128,), np.int64)  # chunk-local idx
                st = consumed.get(c, 0)
                for p in range(128):
                    dl = t * 128 + p
                    a = rp[dl * 4 + c]
                    n = key_counts[dl * 4 + c]
                    seg = s_p[a + st:a + min(n, st + wl)] - c * CH
                    # positions i = col*128 + p
                    k = len(seg)
                    if k:
                        Lbuf[p:(k - 1) * 128 + p + 1:128][:k] = seg
                consumed[c] = st + wl
                Wv = Lbuf.reshape(wl * 8, 16).T.astype(np.int16)
                arr[:, io:io + wl * 8] = np.tile(Wv, (8, 1))
        idx_all.append(arr)

    # ---- x transposed, per-core sorted columns
    xT = np.zeros((F_IN, NP), dtype=np.float32)
    xT[:, :N] = np.asarray(x, dtype=np.float32).T

    W1 = np.asarray(W1, np.float32)
    W2 = np.asarray(W2, np.float32)
    BDs = np.zeros((H1 * C1, H1), np.float32)
    BDd = np.zeros((H1 * C1, H1), np.float32)
    for h in range(H1):
        BDs[h * C1:(h + 1) * C1, h] = np.asarray(att_src1, np.float32)[h]
        BDd[h * C1:(h + 1) * C1, h] = np.asarray(att_dst1, np.float32)[h]
    W1ext = np.concatenate([W1, W1 @ BDs, W1 @ BDd], axis=1)          # [F,80]
    W2ext = np.concatenate(
        [W2, W2 @ np.asarray(att_src2, np.float32).reshape(-1, 1),
         W2 @ np.asarray(att_dst2, np.float32).reshape(-1, 1)], axis=1)

    ident = np.eye(128, dtype=np.float32)

    in_maps = []
    for cidx in range(n_cores):
        in_maps.append({
            "xT_shard": np.ascontiguousarray(xT[:, node_lists[cidx]]),
            "W1ext": W1ext,
            "W2ext": W2ext,
            "b1v": np.asarray(b1, np.float32).reshape(1, -1),
            "b2v": np.asarray(b2, np.float32).reshape(1, -1),
            "idxw": idx_all[cidx],
            "ident": ident,
        })
    meta = dict(N=N, NP=NP, SH=SH, T=T, CH=CH, TOTW=TOTW, K2=K2,
                sched=sched, F_IN=F_IN, H1=H1, C1=C1, C2=C2,
                n_cores=n_cores)
    return in_maps, meta, node_lists


# ------------------------------------------------------------- device build

def build_program(meta, repeat=1):
    NP, SH, T, CH = meta["NP"], meta["SH"], meta["T"], meta["CH"]
    TOTW = meta["TOTW"]
    K2 = meta["K2"]
    sched = meta["sched"]
    F_IN = meta["F_IN"]
    H1, C1, C2 = meta["H1"], meta["C1"], meta["C2"]
    D1 = H1 * C1                   # 64
    n_cores = meta["n_cores"]
    RW = 128                       # table row elems (256B bf16)

    nc = bacc.Bacc("TRN2", target_bir_lowering=False, debug=False,
                   num_devices=n_cores, num_swdge_queues=4)

    xT_d = nc.dram_tensor("xT_shard", [F_IN, SH], F32, kind="ExternalInput")
    W1_d = nc.dram_tensor("W1ext", [F_IN, D1 + 2 * H1], F32, kind="ExternalInput")
    W2_d = nc.dram_tensor("W2ext", [D1, C2 + 2], F32, kind="ExternalInput")
    b1_d = nc.dram_tensor("b1v", [1, D1], F32, kind="ExternalInput")
    b2_d = nc.dram_tensor("b2v", [1, C2], F32, kind="ExternalInput")
    idx_d = nc.dram_tensor("idxw", [128, TOTW], I16, kind="ExternalInput")
    id_d = nc.dram_tensor("ident", [128, 128], F32, kind="ExternalInput")
    out_d = nc.dram_tensor("out", [SH, C2], F32, kind="ExternalOutput")

    t1s_d = nc.dram_tensor("t1_shard", [SH, RW], BF16)
    t1f_d = nc.dram_tensor("t1_full", [NP, RW], BF16, addr_space="Shared")
    ad1_d = nc.dram_tensor("adst1_shard", [SH, H1], F32)
    o1_d = nc.dram_tensor("out1_nat", [SH, D1], F32)
    t2s_d = nc.dram_tensor("t2_shard", [SH, RW], BF16)
    t2f_d = nc.dram_tensor("t2_full", [NP, RW], BF16, addr_space="Shared")
    ad2_d = nc.dram_tensor("adst2_shard", [SH, 1], F32)

    groups = [list(range(n_cores))]
    qrr = [0]

    def next_q():
        q = qrr[0]
        qrr[0] = (q + 1) % 4
        return q

    with tile.TileContext(nc) as tc:
      nc.gpsimd.load_library(library_config.mlp)
      for _rep in range(repeat):
          # ---------------- phase A: L1 projection ----------------
          with tc.tile_pool(name="pa1", bufs=1) as pa1, \
               tc.tile_pool(name="psA", bufs=4, space="PSUM") as psA:
              w1_t = pa1.tile([F_IN, D1 + 2 * H1], F32)
              nc.sync.dma_start(out=w1_t[:], in_=W1_d[:])
              xT_t = pa1.tile([F_IN, SH], F32)
              nc.sync.dma_start(out=xT_t[:], in_=xT_d[:])
              st1 = pa1.tile([128, T, RW], BF16)
              nc.vector.memset(st1[:], 0.0)
              stA = pa1.tile([128, T * H1], F32)
              for t in range(T):
                  ps = psA.tile([128, D1 + 2 * H1], F32, tag="psA")
                  nc.tensor.matmul(ps[:], lhsT=xT_t[:, t * 128:(t + 1) * 128],
                                   rhs=w1_t[:], start=True, stop=True)
                  nc.vector.tensor_copy(out=st1[:, t, 0:D1 + H1],
                                        in_=ps[:, 0:D1 + H1])
                  nc.vector.tensor_copy(out=stA[:, t * H1:(t + 1) * H1],
                                        in_=ps[:, D1 + H1:D1 + 2 * H1])
              nc.sync.dma_start(
                  out=t1s_d.ap().rearrange("(t p) c -> p t c", p=128),
                  in_=st1[:])
              nc.sync.dma_start(
                  out=ad1_d.ap().rearrange("(t p) c -> p t c", p=128),
                  in_=stA[:].rearrange("p (t c) -> p t c", c=H1))

          nc.gpsimd.collective_compute(
              "AllGather", OP.bypass, replica_groups=groups,
              ins=[t1s_d[:]], outs=[t1f_d[:]])

          with tc.tile_pool(name="poi", bufs=1) as poi:
              pz = poi.tile([1, H1], BF16)
              nc.vector.memset(pz[:], POISON)
              for c in range(4):
                  nc.sync.dma_start(
                      out=t1f_d[c * CH:c * CH + 1, D1:D1 + H1], in_=pz[:])

          # ---------------- phase B: L1 edge aggregation ----------------
          with tc.tile_pool(name="pix", bufs=1) as pix:
            idx_t = pix.tile([128, TOTW], I16)
            nc.sync.dma_start(out=idx_t[:], in_=idx_d[:])

            with tc.tile_pool(name="pb", bufs=3) as pb, \
                 tc.tile_pool(name="pb1", bufs=1) as pb1:
              ad1_all = pb1.tile([128, T, H1], F32)
              nc.sync.dma_start(out=ad1_all[:],
                                in_=ad1_d.ap().rearrange("(t p) c -> p t c", p=128))
              o1_st = pb1.tile([128, T * D1], F32)
              b1_t = pb1.tile([128, D1], F32)
              nc.sync.dma_start(out=b1_t[:], in_=b1_d[:].to_broadcast([128, D1]))

              for t in range(T):
                  ent, K = sched[t], K2[t]
                  g = pb.tile([128, K, RW], BF16, tag="g1")
                  for (c, wl, io, go) in ent[0]:
                      nc.gpsimd.dma_gather(
                          out_ap=g[:, go:go + wl, :],
                          in_ap=t1f_d[c * CH:(c + 1) * CH, :],
                          idxs_ap=idx_t[:, io:io + wl * 8],
                          num_idxs=wl * 128, num_idxs_reg=wl * 128,
                          elem_size=RW, queue_num=next_q())
                  adst = ad1_all[:, t, :]
                  asf = pb.tile([128, K, H1], F32, tag="as1")
                  nc.vector.tensor_copy(out=asf[:], in_=g[:, :, D1:D1 + H1])
                  lg = pb.tile([128, K, H1], F32, tag="lg1")
                  nc.vector.tensor_tensor(
                      out=lg[:], in0=asf[:],
                      in1=adst.unsqueeze(1).to_broadcast([128, K, H1]),
                      op=OP.add)
                  nc.vector.scalar_tensor_tensor(
                      out=lg[:], in0=lg[:], scalar=NEG_SLOPE, in1=lg[:],
                      op0=OP.mult, op1=OP.max)
                  w = pb.tile([128, K, H1], BF16, tag="w1")
                  nc.scalar.activation(w[:], lg[:], AF.Exp)
                  gh = g[:, :, 0:D1].rearrange("p k (h c) -> p k h c", c=C1)
                  nc.vector.tensor_tensor(
                      out=gh, in0=gh,
                      in1=w[:].unsqueeze(3).to_broadcast([128, K, H1, C1]),
                      op=OP.mult)
                  S = pb.tile([128, D1], F32, tag="S1")
                  nc.vector.tensor_reduce(
                      out=S[:], in_=g[:, :, 0:D1].rearrange("p k c -> p c k"),
                      axis=mybir.AxisListType.X, op=OP.add)
                  z = pb.tile([128, H1], F32, tag="z1")
                  nc.vector.tensor_reduce(
                      out=z[:], in_=w[:].rearrange("p k h -> p h k"),
                      axis=mybir.AxisListType.X, op=OP.add)
                  nc.vector.tensor_scalar_add(z[:], z[:], EPS)
                  rz = pb.tile([128, H1], F32, tag="rz1")
                  nc.vector.reciprocal(rz[:], z[:])
                  o = pb.tile([128, D1], F32, tag="o1")
                  nc.vector.tensor_tensor(
                      out=o[:].rearrange("p (h c) -> p h c", c=C1),
                      in0=S[:].rearrange("p (h c) -> p h c", c=C1),
                      in1=rz[:].unsqueeze(2).to_broadcast([128, H1, C1]),
                      op=OP.mult)
                  nc.vector.tensor_tensor(
                      out=o[:], in0=o[:], in1=b1_t[:], op=OP.add)
                  tmin = pb.tile([128, D1], F32, tag="tm1")
                  nc.vector.tensor_scalar_min(tmin[:], o[:], 0.0)
                  texp = pb.tile([128, D1], F32, tag="te1")
                  nc.scalar.activation(texp[:], tmin[:], AF.Exp)
                  nc.vector.tensor_scalar_max(o[:], o[:], 0.0)
                  nc.vector.scalar_tensor_tensor(
                      out=o1_st[:, t * D1:(t + 1) * D1], in0=texp[:],
                      scalar=-1.0, in1=o[:], op0=OP.add, op1=OP.add)
              nc.sync.dma_start(
                  out=o1_d.ap().rearrange("(t p) c -> p t c", p=128),
                  in_=o1_st[:].rearrange("p (t c) -> p t c", c=D1))

            # ---------------- phase A2: L2 projection ----------------
            with tc.tile_pool(name="pc", bufs=3) as pc, \
                 tc.tile_pool(name="pc1", bufs=1) as pc1, \
                 tc.tile_pool(name="psC", bufs=4, space="PSUM") as psC:
                id_t = pc1.tile([128, 128], F32)
                nc.sync.dma_start(out=id_t[:], in_=id_d[:])
                w2_t = pc1.tile([D1, C2 + 2], F32)
                nc.sync.dma_start(out=w2_t[:], in_=W2_d[:])
                st2 = pc1.tile([128, T, RW], BF16)
                nc.vector.memset(st2[:], 0.0)
                stA2 = pc1.tile([128, T], F32)
                for t in range(T):
                    h = pc.tile([128, D1], F32, tag="h1n")
                    nc.sync.dma_start(out=h[:], in_=o1_d[t * 128:(t + 1) * 128, :])
                    pst = psC.tile([D1, 128], F32, tag="psT")
                    nc.tensor.transpose(pst[:], h[:], id_t[:])
                    hT = pc.tile([D1, 128], F32, tag="hT")
                    nc.vector.tensor_copy(out=hT[:], in_=pst[:])
                    ps2 = psC.tile([128, C2 + 2], F32, tag="ps2")
                    nc.tensor.matmul(ps2[:], lhsT=hT[:], rhs=w2_t[:],
                                     start=True, stop=True)
                    nc.vector.tensor_copy(out=st2[:, t, 0:C2 + 1],
                                          in_=ps2[:, 0:C2 + 1])
                    nc.vector.tensor_copy(out=stA2[:, t:t + 1],
                                          in_=ps2[:, C2 + 1:C2 + 2])
                nc.sync.dma_start(
                    out=t2s_d.ap().rearrange("(t p) c -> p t c", p=128),
                    in_=st2[:])
                nc.sync.dma_start(
                    out=ad2_d.ap().rearrange("(t p) c -> p t c", p=128),
                    in_=stA2[:].unsqueeze(2))

            nc.gpsimd.collective_compute(
                "AllGather", OP.bypass, replica_groups=groups,
                ins=[t2s_d[:]], outs=[t2f_d[:]])

            with tc.tile_pool(name="poi2", bufs=1) as poi2:
                pz2 = poi2.tile([1, 1], BF16)
                nc.vector.memset(pz2[:], POISON)
                for c in range(4):
                    nc.sync.dma_start(
                        out=t2f_d[c * CH:c * CH + 1, C2:C2 + 1], in_=pz2[:])

            # ---------------- phase C: L2 edge + log_softmax ----------------
            with tc.tile_pool(name="pd", bufs=3) as pd, \
                 tc.tile_pool(name="pd1", bufs=1) as pd1:
              ad2_all = pd1.tile([128, T, 1], F32)
              nc.sync.dma_start(out=ad2_all[:],
                                in_=ad2_d.ap().rearrange("(t p) c -> p t c", p=128))
              o2_st = pd1.tile([128, T * C2], F32)
              b2_t = pd1.tile([128, C2], F32)
              nc.sync.dma_start(out=b2_t[:], in_=b2_d[:].to_broadcast([128, C2]))

              for t in range(T):
                  ent, K = sched[t], K2[t]
                  g2 = pd.tile([128, K, RW], BF16, tag="g2")
                  for (c, wl, io, go) in ent[0]:
                      nc.gpsimd.dma_gather(
                          out_ap=g2[:, go:go + wl, :],
                          in_ap=t2f_d[c * CH:(c + 1) * CH, :],
                          idxs_ap=idx_t[:, io:io + wl * 8],
                          num_idxs=wl * 128, num_idxs_reg=wl * 128,
                          elem_size=RW, queue_num=next_q())
                  ad2 = ad2_all[:, t, :]
                  as2 = pd.tile([128, K], F32, tag="as2")
                  nc.vector.tensor_copy(out=as2[:], in_=g2[:, :, C2])
                  lg2 = pd.tile([128, K], F32, tag="lg2")
                  nc.vector.tensor_tensor(
                      out=lg2[:], in0=as2[:],
                      in1=ad2.to_broadcast([128, K]), op=OP.add)
                  nc.vector.scalar_tensor_tensor(
                      out=lg2[:], in0=lg2[:], scalar=NEG_SLOPE, in1=lg2[:],
                      op0=OP.mult, op1=OP.max)
                  w2 = pd.tile([128, K], BF16, tag="w2")
                  nc.scalar.activation(w2[:], lg2[:], AF.Exp)
                  nc.vector.tensor_tensor(
                      out=g2[:, :, 0:C2], in0=g2[:, :, 0:C2],
                      in1=w2[:].unsqueeze(2).to_broadcast([128, K, C2]),
                      op=OP.mult)
                  S2 = pd.tile([128, C2], F32, tag="S2")
                  nc.vector.tensor_reduce(
                      out=S2[:], in_=g2[:, :, 0:C2].rearrange("p k c -> p c k"),
                      axis=mybir.AxisListType.X, op=OP.add)
                  z2 = pd.tile([128, 1], F32, tag="z2")
                  nc.vector.tensor_reduce(
                      out=z2[:], in_=w2[:].unsqueeze(1),
                      axis=mybir.AxisListType.X, op=OP.add)
                  nc.vector.tensor_scalar_add(z2[:], z2[:], EPS)
                  rz2 = pd.tile([128, 1], F32, tag="rz2")
                  nc.vector.reciprocal(rz2[:], z2[:])
                  o2 = pd.tile([128, C2], F32, tag="o2")
                  nc.vector.tensor_tensor(
                      out=o2[:], in0=S2[:],
                      in1=rz2[:].to_broadcast([128, C2]), op=OP.mult)
                  nc.vector.tensor_tensor(
                      out=o2[:], in0=o2[:], in1=b2_t[:], op=OP.add)
                  mx = pd.tile([128, 1], F32, tag="mx")
                  nc.vector.tensor_reduce(out=mx[:], in_=o2[:],
                                          axis=mybir.AxisListType.X, op=OP.max)
                  nc.vector.tensor_scalar(
                      out=o2[:], in0=o2[:], scalar1=mx[:], scalar2=None,
                      op0=OP.subtract)
                  ex = pd.tile([128, C2], F32, tag="ex")
                  nc.scalar.activation(ex[:], o2[:], AF.Exp)
                  sz = pd.tile([128, 1], F32, tag="sz")
                  nc.vector.tensor_reduce(out=sz[:], in_=ex[:],
                                          axis=mybir.AxisListType.X, op=OP.add)
                  lnz = pd.tile([128, 1], F32, tag="lnz")
                  nc.scalar.activation(lnz[:], sz[:], AF.Ln)
                  nc.vector.tensor_scalar(
                      out=o2_st[:, t * C2:(t + 1) * C2], in0=o2[:],
                      scalar1=lnz[:], scalar2=None, op0=OP.subtract)
              nc.sync.dma_start(
                  out=out_d.ap().rearrange("(t p) c -> p t c", p=128),
                  in_=o2_st[:].rearrange("p (t c) -> p t c", c=C2))

    nc.compile()
    return nc


# ------------------------------------------------------------- public entry

def _make_runner(nc, n_cores):
    """Jitted shard_map callable for a prebuilt Bass module (self-contained)."""
    import jax
    from jax.sharding import Mesh, PartitionSpec
    from jax.experimental.shard_map import shard_map
    from concourse import bass2jax
    from concourse.bass2jax import _bass_exec_p, partition_id_tensor

    bass2jax.install_neuronx_cc_hook()
    partition_name = nc.partition_id_tensor.name if nc.partition_id_tensor else None
    in_names, out_names, out_avals, zero_outs = [], [], [], []
    for alloc in nc.m.functions[0].allocations:
        if not isinstance(alloc, mybir.MemoryLocationSet):
            continue
        name = alloc.memorylocations[0].name
        if alloc.kind == "ExternalInput":
            if name != partition_name:
                in_names.append(name)
        elif alloc.kind == "ExternalOutput":
            out_names.append(name)
            shape = tuple(alloc.tensor_shape)
            dtype = mybir.dt.np(alloc.dtype)
            out_avals.append(jax.core.ShapedArray(shape, dtype))
            zero_outs.append(np.zeros(shape, dtype))
    n_params = len(in_names)
    n_outs = len(out_avals)
    all_in_names = list(in_names) + list(out_names)
    if partition_name is not None:
        all_in_names.append(partition_name)

    def _body(*args):
        operands = list(args)
        if partition_name is not None:
            operands.append(partition_id_tensor())
        outs = _bass_exec_p.bind(
            *operands,
            out_avals=tuple(out_avals),
            in_names=tuple(all_in_names),
            out_names=tuple(out_names),
            lowering_input_output_aliases=(),
            sim_require_finite=True,
            sim_require_nnan=True,
            nc=nc,
        )
        return tuple(outs)

    devices = jax.devices()[:n_cores]
    mesh = Mesh(np.asarray(devices), ("core",))
    in_specs = (PartitionSpec("core"),) * (n_params + n_outs)
    out_specs = (PartitionSpec("core"),) * n_outs
    sharded = jax.jit(
        shard_map(_body, mesh=mesh, in_specs=in_specs, out_specs=out_specs,
                  check_rep=False),
        keep_unused=True,
    )

    def stage(in_maps):
        import jax
        concat_in = [
            np.concatenate([np.asarray(in_maps[c][nm]) for c in range(n_cores)], 0)
            for nm in in_names
        ] + [np.concatenate([z] * n_cores, 0) for z in zero_outs]
        return [jax.device_put(a) for a in concat_in]

    return sharded, stage, out_names, out_avals



_CACHE = {}


def _runner_for(meta):
    key = (meta["NP"], meta["TOTW"], tuple(meta["K2"]))
    if key in _CACHE:
        return _CACHE[key]
    nc = build_program(meta)
    n_cores = meta["n_cores"]
    sharded, stage, out_names, out_avals = _make_runner(nc, n_cores)

    def run(in_maps):
        import jax
        staged = stage(in_maps)
        outs = sharded(*staged)
        jax.block_until_ready(outs)
        outs_np = [np.asarray(o) for o in outs]
        results = []
        for c in range(n_cores):
            m = {}
            for i, nm in enumerate(out_names):
                sh = out_avals[i].shape
                m[nm] = outs_np[i][c * sh[0]:(c + 1) * sh[0]]
            results.append(m)
        return results

    _CACHE[key] = run
    return run


def kernel(x, edge_index, W1, att_src1, att_dst1, b1, W2, att_src2,
           att_dst2, b2):
    """Full-input GAT forward on 8 NeuronCores; returns [N, C2] float32."""
    x = np.asarray(x)
    N = x.shape[0]
    in_maps, meta, node_lists = host_prep(
        x, edge_index, W1, att_src1, att_dst1, b1, W2, att_src2, att_dst2,
        b2, n_cores=8)
    run = _runner_for(meta)
    results = run(in_maps)
    C2 = meta["C2"]
    nat = np.empty((meta["NP"], C2), np.float32)
    for c in range(meta["n_cores"]):
        nat[node_lists[c]] = results[c]["out"]
    return np.ascontiguousarray(nat[:N], dtype=np.float32)


# revision 7
# speedup vs baseline: 2.0186x; 1.2100x over previous
"""GAT kernel for TRN2, v2: dma_gather (SWDGE Ant ucode) edge gathers.

Design (vs v1's per-column indirect1d):
- Gather primitive: gpsimd.dma_gather, bf16 256B table rows, 16 idx/desc,
  ~2.8ns/idx at 8 cores (vs ~11ns/edge for indirect1d).
- int16 idx limit (<32768 rows) handled by splitting the node table into 4
  contiguous chunks = core pairs {0,1},{2,3},{4,5},{6,7} (2*SH<=32768 rows).
- Per-dst chunk balance: nodes are assigned to cores by a batched greedy
  that balances each dst's in-edges across the 4 chunks, so per-(tile,chunk)
  ELL widths ~= ceil(K/4) and idx inflation stays ~15-25%.
- Landing layout (transpose=False): idx i -> (partition i%128, col i//128),
  i.e. column-major slot grid per 128-dst tile — same compute structure as
  v1 (logits from cols 64:72, messages cols 0:64, per-partition reduces).
- Tables: T1 [NP,128]bf16 = [h1(64)|a_src1(8)|pad]; T2 [NP,128]bf16 =
  [h2(16)|a_src2(1)|pad]. a_dst kept per-shard f32. AllGather between
  layers. Chunk-base rows are pad nodes; their a_src is poisoned to -1e30
  after each AllGather so pad slots contribute exactly 0.
"""
import numpy as np

import concourse.bacc as bacc
import concourse.bass as bass
import concourse.mybir as mybir
import concourse.tile as tile
from concourse import library_config

F32 = mybir.dt.float32
BF16 = mybir.dt.bfloat16
I16 = mybir.dt.int16
AF = mybir.ActivationFunctionType
OP = mybir.AluOpType

NEG_SLOPE = 0.2
EPS = 1e-16
POISON = -1.0e30
MAXW = 8                      # max ELL columns per dma_gather (NI<=1024)


# ---------------------------------------------------------------- host prep

def _ranges(counts):
    """[0..c0), [0..c1), ... concatenated."""
    total = int(counts.sum())
    out = np.arange(total, dtype=np.int64)
    starts = np.zeros(len(counts), np.int64)
    np.cumsum(counts[:-1], out=starts[1:])
    return out - np.repeat(starts, counts)


def _assign_classes(src, dst, N, SH, rng):
    """Greedy balanced 4-coloring of nodes: per dst, spread its in-edge srcs
    evenly over the 4 classes. Returns cls[N] in 0..3."""
    deg = np.bincount(dst, minlength=N)
    cap = -(-deg // 4)
    cnt = np.zeros((N, 4), np.int32)
    order_e = np.argsort(src, kind="stable")
    s_sorted = src[order_e]
    d_sorted = dst[order_e]
    rowptr = np.zeros(N + 1, np.int64)
    np.cumsum(np.bincount(s_sorted, minlength=N), out=rowptr[1:])
    n_pad = 8 * SH - N
    classcap = 2 * SH - 2 * (n_pad // 8)   # real-node capacity per class
    fill = np.zeros(4, np.int64)
    cls = np.full(N, -1, np.int8)
    perm = rng.permutation(N)
    B = 4096
    for i in range(0, N, B):
        batch = perm[i:i + B]
        counts = (rowptr[batch + 1] - rowptr[batch]).astype(np.int64)
        flat = np.repeat(rowptr[batch], counts) + _ranges(counts)
        dd = d_sorted[flat]
        owner = np.repeat(np.arange(len(batch)), counts)
        over = cnt[dd].astype(np.float64)
        over += 1e3 * (cnt[dd] >= cap[dd][:, None])             # [Eb, 4]
        sc = np.zeros((len(batch), 4), np.float64)
        np.add.at(sc, owner, over)
        sc += rng.random(sc.shape) * 0.01
        full_cls = fill >= classcap
        sc[:, full_cls] = np.inf
        c_star = np.argmin(sc, axis=1)
        for _ in range(3):        # quota repair
            done = True
            for c in range(4):
                sel = np.where(c_star == c)[0]
                overflow = int(fill[c] + len(sel) - classcap)
                if overflow > 0:
                    done = False
                    move = sel[np.argsort(sc[sel, c])[::-1][:overflow]]
                    sc[move, c] = np.inf
                    c_star[move] = np.argmin(sc[move], axis=1)
            if done:
                break
        cls[batch] = c_star.astype(np.int8)
        fill += np.bincount(c_star, minlength=4)
        np.add.at(cnt, (dd, c_star[owner]), 1)

    # iterative swap repair guided by the true cost: sum over (tile,chunk)
    # of the max in-class count over the tile's 128 dsts (max over cores).
    n_cores = 8
    pads_per_core = n_pad // 8
    real_cap = SH - pads_per_core

    def true_cost(cls_v):
        core_of = np.full(N, -1, np.int64)
        for c4 in range(4):
            nodes = np.where(cls_v == c4)[0]
            core_of[nodes[:real_cap]] = 2 * c4
            core_of[nodes[real_cap:]] = 2 * c4 + 1
        tot = None
        T = SH // 128
        for cidx in range(n_cores):
            nodes = np.where(core_of == cidx)[0]
            order = np.argsort(deg[nodes], kind="stable")
            pos = np.empty(len(nodes), np.int64)
            pos[order] = np.arange(len(nodes))
            posl = np.full(N, -1, np.int64)
            posl[nodes] = pos
            m = core_of[dst] == cidx
            npads = SH - len(nodes)
            d_loc = posl[dst[m]] + npads
            key = d_loc * 4 + cls_v[src[m]]
            cnts = np.bincount(key, minlength=SH * 4).reshape(T, 128, 4)
            w = cnts.max(axis=1)
            tot = w if tot is None else np.maximum(tot, w)
        return int(np.maximum(tot, 0).sum()), tot

    best_cost, _ = true_cost(cls)
    best_cls = cls.copy()
    for _ in range(8):
        cv = cls[s_sorted].astype(np.int64)
        lose_e = np.maximum(
            2 * (cnt[d_sorted, cv] - cap[d_sorted]) - 1, 0).astype(np.float64)
        lose = np.zeros(N)
        np.add.at(lose, s_sorted, lose_e)
        gain_e = np.maximum(
            2 * (cnt[d_sorted] - cap[d_sorted][:, None]) + 1, 0).astype(np.float64)
        gain = np.zeros((N, 4))
        np.add.at(gain, s_sorted, gain_e)
        gain[np.arange(N), cls.astype(np.int64)] = np.inf
        best_c2 = np.argmin(gain, axis=1)
        score = lose - gain[np.arange(N), best_c2]
        cand = np.where(score > 0)[0]
        if len(cand) < 16:
            break
        cand = cand[rng.random(len(cand)) < 0.3]
        mdict = {}
        for a in range(4):
            for b in range(4):
                if a != b:
                    vs = cand[(cls[cand] == a) & (best_c2[cand] == b)]
                    mdict[(a, b)] = vs[np.argsort(-score[vs])]
        moved = []
        for a in range(4):
            for b in range(a + 1, 4):
                va, vb = mdict[(a, b)], mdict[(b, a)]
                k = min(len(va), len(vb))
                if k:
                    moved.append((va[:k], b))
                    moved.append((vb[:k], a))
        if not moved:
            break
        for vs, newc in moved:
            old_c = cls[vs].astype(np.int64)
            cls[vs] = newc
            counts_v = rowptr[vs + 1] - rowptr[vs]
            flat = np.repeat(rowptr[vs], counts_v) + _ranges(counts_v)
            dd2 = d_sorted[flat]
            np.add.at(cnt, (dd2, np.repeat(old_c, counts_v)), -1)
            np.add.at(cnt, (dd2, np.full(int(counts_v.sum()), newc)), 1)
        cost, _ = true_cost(cls)
        if cost < best_cost:
            best_cost = cost
            best_cls = cls.copy()
    cls = best_cls
    return cls


def host_prep(x, edge_index, W1, att_src1, att_dst1, b1, W2, att_src2,
              att_dst2, b2, n_cores=8):
    N = x.shape[0]
    F_IN = x.shape[1]
    H1, C1 = att_src1.shape
    C2 = att_src2.shape[1]
    SH = -(-N // (128 * n_cores)) * 128
    NP = SH * n_cores
    T = SH // 128
    CH = 2 * SH                       # chunk rows
    rng = np.random.default_rng(12345)

    src = np.concatenate([np.asarray(edge_index[0]), np.arange(N)]).astype(np.int64)
    dst = np.concatenate([np.asarray(edge_index[1]), np.arange(N)]).astype(np.int64)

    # ---- balanced class assignment; class c -> cores {2c, 2c+1}
    cls = _assign_classes(src, dst, N, SH, rng)
    n_pad = NP - N
    pads_per_core = n_pad // n_cores
    real_cap = SH - pads_per_core
    core_of_node = np.full(NP, -1, np.int64)
    # nodes of class c fill core 2c then 2c+1
    for c in range(4):
        nodes = np.where(cls == c)[0]
        assert len(nodes) <= 2 * real_cap, (c, len(nodes), real_cap)
        core_of_node[nodes[:real_cap]] = 2 * c
        core_of_node[nodes[real_cap:]] = 2 * c + 1
    # pads fill remaining capacity
    pad_ids = np.arange(N, NP)
    counts = np.bincount(core_of_node[:N][core_of_node[:N] >= 0],
                         minlength=n_cores)
    pp = 0
    for cidx in range(n_cores):
        k = SH - int(counts[cidx])
        core_of_node[pad_ids[pp:pp + k]] = cidx
        pp += k
    assert pp == n_pad

    # ---- per-core degree sort (pads/deg0 first), posmap
    deg_all = np.bincount(dst, minlength=NP)
    perms = []          # natural node ids (local index) in sorted order
    node_lists = []
    posmap = np.zeros(NP, dtype=np.int64)
    for cidx in range(n_cores):
        nodes = np.where(core_of_node == cidx)[0]
        order = np.argsort(deg_all[nodes], kind="stable")
        snodes = nodes[order]
        node_lists.append(snodes)
        posmap[snodes] = cidx * SH + np.arange(SH)
    # each chunk's base row must be a zero-degree pad (poison target)
    for c in range(4):
        base_node = node_lists[2 * c][0]
        assert deg_all[base_node] == 0 and base_node >= N, "chunk base not pad"

    # ---- per-core ELL grids split by class
    src_pos = posmap[src]
    dst_core = core_of_node[dst]
    sched = None                       # [(t, [(cls, wlen, ioff, goff)...])]
    Wtc = np.zeros((n_cores, T, 4), np.int64)
    grids = []                         # per core: [T][4] -> [128, w] local idx
    for cidx in range(n_cores):
        m = dst_core == cidx
        s_p = src_pos[m]
        d_loc = posmap[dst[m]] - cidx * SH
        order = np.lexsort((s_p // CH, d_loc))     # by dst, then class
        s_p = s_p[order]
        d_loc = d_loc[order]
        e_cls = (s_p // CH).astype(np.int64)
        # CSR boundaries per (dst, class)
        key = d_loc * 4 + e_cls
        cnts = np.bincount(key, minlength=SH * 4).reshape(SH, 4)
        Wtc[cidx] = cnts.reshape(T, 128, 4).max(axis=1)
        grids.append((s_p, d_loc, e_cls, cnts))

    W_eq = Wtc.max(axis=0)             # [T, 4] equalized widths
    # schedule: per tile, windows of <= MAXW columns
    sched = []
    ioff = 0
    for t in range(T):
        ent = []
        goff = 0
        for c in range(4):
            w = int(W_eq[t, c])
            if t == 0 and c == 0 and W_eq.sum() == 0:
                w = 1
            while w > 0:
                wl = min(w, MAXW)
                # balance the split (9 -> 5+4 not 8+1)
                nwin = -(-w // MAXW)
                wl = -(-w // nwin)
                ent.append((c, wl, ioff, goff))
                ioff += wl * 8        # idx cols (128*wl/16)
                goff += wl
                w -= wl
        if goff == 0:                  # fully empty tile: one pad window
            ent.append((0, 1, ioff, 0))
            ioff += 8
            goff = 1
        sched.append((ent, goff))
    TOTW = ioff
    K2 = [s[1] for s in sched]

    # ---- idx tensors per core
    idx_all = []
    for cidx in range(n_cores):
        s_p, d_loc, e_cls, cnts = grids[cidx]
        # rowptr over (dst, class)
        key_counts = cnts.reshape(-1)
        rp = np.zeros(SH * 4 + 1, np.int64)
        np.cumsum(key_counts, out=rp[1:])
        arr = np.zeros((128, TOTW), np.int16)
        for t in range(T):
            ent, _ = sched[t]
            consumed = {}
            for (c, wl, io, go) in ent:
                Lbuf = np.zeros((wl * 128,), np.int64)  # chunk-local idx
                st = consumed.get(c, 0)
                for p in range(128):
                    dl = t * 128 + p
                    a = rp[dl * 4 + c]
                    n = key_counts[dl * 4 + c]
                    seg = s_p[a + st:a + min(n, st + wl)] - c * CH
                    # positions i = col*128 + p
                    k = len(seg)
                    if k:
                        Lbuf[p:(k - 1) * 128 + p + 1:128][:k] = seg
                consumed[c] = st + wl
                Wv = Lbuf.reshape(wl * 8, 16).T.astype(np.int16)
                arr[:, io:io + wl * 8] = np.tile(Wv, (8, 1))
        idx_all.append(arr)

    # ---- x transposed, per-core sorted columns
    xT = np.zeros((F_IN, NP), dtype=np.float32)
    xT[:, :N] = np.asarray(x, dtype=np.float32).T

    W1 = np.asarray(W1, np.float32)
    W2 = np.asarray(W2, np.float32)
    BDs = np.zeros((H1 * C1, H1), np.float32)
    BDd = np.zeros((H1 * C1, H1), np.float32)
    for h in range(H1):
        BDs[h * C1:(h + 1) * C1, h] = np.asarray(att_src1, np.float32)[h]
        BDd[h * C1:(h + 1) * C1, h] = np.asarray(att_dst1, np.float32)[h]
    W1ext = np.concatenate([W1, W1 @ BDs, W1 @ BDd], axis=1)          # [F,80]
    W2ext = np.concatenate(
        [W2, W2 @ np.asarray(att_src2, np.float32).reshape(-1, 1),
         W2 @ np.asarray(att_dst2, np.float32).reshape(-1, 1)], axis=1)

    ident = np.eye(128, dtype=np.float32)

    in_maps = []
    for cidx in range(n_cores):
        in_maps.append({
            "xT_shard": np.ascontiguousarray(xT[:, node_lists[cidx]]),
            "W1ext": W1ext,
            "W2ext": W2ext,
            "b1v": np.asarray(b1, np.float32).reshape(1, -1),
            "b2v": np.asarray(b2, np.float32).reshape(1, -1),
            "idxw": idx_all[cidx],
            "ident": ident,
        })
    meta = dict(N=N, NP=NP, SH=SH, T=T, CH=CH, TOTW=TOTW, K2=K2,
                sched=sched, F_IN=F_IN, H1=H1, C1=C1, C2=C2,
                n_cores=n_cores)
    return in_maps, meta, node_lists


# ------------------------------------------------------------- device build

def build_program(meta, repeat=1):
    NP, SH, T, CH = meta["NP"], meta["SH"], meta["T"], meta["CH"]
    TOTW = meta["TOTW"]
    K2 = meta["K2"]
    sched = meta["sched"]
    F_IN = meta["F_IN"]
    H1, C1, C2 = meta["H1"], meta["C1"], meta["C2"]
    D1 = H1 * C1                   # 64
    n_cores = meta["n_cores"]
    RW = 128                       # table row elems (256B bf16)

    nc = bacc.Bacc("TRN2", target_bir_lowering=False, debug=False,
                   num_devices=n_cores, num_swdge_queues=4)

    xT_d = nc.dram_tensor("xT_shard", [F_IN, SH], F32, kind="ExternalInput")
    W1_d = nc.dram_tensor("W1ext", [F_IN, D1 + 2 * H1], F32, kind="ExternalInput")
    W2_d = nc.dram_tensor("W2ext", [D1, C2 + 2], F32, kind="ExternalInput")
    b1_d = nc.dram_tensor("b1v", [1, D1], F32, kind="ExternalInput")
    b2_d = nc.dram_tensor("b2v", [1, C2], F32, kind="ExternalInput")
    idx_d = nc.dram_tensor("idxw", [128, TOTW], I16, kind="ExternalInput")
    id_d = nc.dram_tensor("ident", [128, 128], F32, kind="ExternalInput")
    out_d = nc.dram_tensor("out", [SH, C2], F32, kind="ExternalOutput")

    t1s_d = nc.dram_tensor("t1_shard", [SH, RW], BF16)
    t1f_d = nc.dram_tensor("t1_full", [NP, RW], BF16, addr_space="Shared")
    ad1_d = nc.dram_tensor("adst1_shard", [SH, H1], F32)
    o1_d = nc.dram_tensor("out1_nat", [SH, D1], F32)
    t2s_d = nc.dram_tensor("t2_shard", [SH, RW], BF16)
    t2f_d = nc.dram_tensor("t2_full", [NP, RW], BF16, addr_space="Shared")
    ad2_d = nc.dram_tensor("adst2_shard", [SH, 1], F32)

    groups = [list(range(n_cores))]
    qrr = [0]

    def next_q():
        q = qrr[0]
        qrr[0] = (q + 1) % 4
        return q

    with tile.TileContext(nc) as tc:
      nc.gpsimd.load_library(library_config.mlp)
      for _rep in range(repeat):
          # ---------------- phase A: L1 projection ----------------
          with tc.tile_pool(name="pa1", bufs=1) as pa1, \
               tc.tile_pool(name="psA", bufs=4, space="PSUM") as psA:
              w1_t = pa1.tile([F_IN, D1 + 2 * H1], F32)
              nc.sync.dma_start(out=w1_t[:], in_=W1_d[:])
              xT_t = pa1.tile([F_IN, SH], F32)
              nc.sync.dma_start(out=xT_t[:], in_=xT_d[:])
              st1 = pa1.tile([128, T, RW], BF16)
              nc.vector.memset(st1[:], 0.0)
              stA = pa1.tile([128, T * H1], F32)
              for t in range(T):
                  ps = psA.tile([128, D1 + 2 * H1], F32, tag="psA")
                  nc.tensor.matmul(ps[:], lhsT=xT_t[:, t * 128:(t + 1) * 128],
                                   rhs=w1_t[:], start=True, stop=True)
                  nc.vector.tensor_copy(out=st1[:, t, 0:D1 + H1],
                                        in_=ps[:, 0:D1 + H1])
                  nc.vector.tensor_copy(out=stA[:, t * H1:(t + 1) * H1],
                                        in_=ps[:, D1 + H1:D1 + 2 * H1])
              nc.sync.dma_start(
                  out=t1s_d.ap().rearrange("(t p) c -> p t c", p=128),
                  in_=st1[:])
              nc.sync.dma_start(
                  out=ad1_d.ap().rearrange("(t p) c -> p t c", p=128),
                  in_=stA[:].rearrange("p (t c) -> p t c", c=H1))

          nc.gpsimd.collective_compute(
              "AllGather", OP.bypass, replica_groups=groups,
              ins=[t1s_d[:]], outs=[t1f_d[:]])

          with tc.tile_pool(name="poi", bufs=1) as poi:
              pz = poi.tile([1, H1], BF16)
              nc.vector.memset(pz[:], POISON)
              for c in range(4):
                  nc.sync.dma_start(
                      out=t1f_d[c * CH:c * CH + 1, D1:D1 + H1], in_=pz[:])

          # ---------------- phase B: L1 edge aggregation ----------------
          with tc.tile_pool(name="pix", bufs=1) as pix:
            idx_t = pix.tile([128, TOTW], I16)
            nc.sync.dma_start(out=idx_t[:], in_=idx_d[:])

            with tc.tile_pool(name="pb", bufs=4) as pb, \
                 tc.tile_pool(name="pb1", bufs=1) as pb1:
              ad1_all = pb1.tile([128, T, H1], F32)
              nc.sync.dma_start(out=ad1_all[:],
                                in_=ad1_d.ap().rearrange("(t p) c -> p t c", p=128))
              o1_st = pb1.tile([128, T * D1], F32)
              b1_t = pb1.tile([128, D1], F32)
              nc.sync.dma_start(out=b1_t[:], in_=b1_d[:].to_broadcast([128, D1]))

              for t in range(T):
                  ent, K = sched[t], K2[t]
                  g = pb.tile([128, K, RW], BF16, tag="g1")
                  for (c, wl, io, go) in ent[0]:
                      nc.gpsimd.dma_gather(
                          out_ap=g[:, go:go + wl, :],
                          in_ap=t1f_d[c * CH:(c + 1) * CH, :],
                          idxs_ap=idx_t[:, io:io + wl * 8],
                          num_idxs=wl * 128, num_idxs_reg=wl * 128,
                          elem_size=RW, queue_num=next_q())
                  adst = ad1_all[:, t, :]
                  asf = pb.tile([128, K, H1], F32, tag="as1")
                  nc.vector.tensor_copy(out=asf[:], in_=g[:, :, D1:D1 + H1])
                  lg = pb.tile([128, K, H1], F32, tag="lg1")
                  nc.vector.tensor_tensor(
                      out=lg[:], in0=asf[:],
                      in1=adst.unsqueeze(1).to_broadcast([128, K, H1]),
                      op=OP.add)
                  nc.vector.scalar_tensor_tensor(
                      out=lg[:], in0=lg[:], scalar=NEG_SLOPE, in1=lg[:],
                      op0=OP.mult, op1=OP.max)
                  w = pb.tile([128, K, H1], BF16, tag="w1")
                  nc.scalar.activation(w[:], lg[:], AF.Exp)
                  gh = g[:, :, 0:D1].rearrange("p k (h c) -> p k h c", c=C1)
                  nc.vector.tensor_tensor(
                      out=gh, in0=gh,
                      in1=w[:].unsqueeze(3).to_broadcast([128, K, H1, C1]),
                      op=OP.mult)
                  S = pb.tile([128, D1], F32, tag="S1")
                  nc.vector.tensor_reduce(
                      out=S[:], in_=g[:, :, 0:D1].rearrange("p k c -> p c k"),
                      axis=mybir.AxisListType.X, op=OP.add)
                  z = pb.tile([128, H1], F32, tag="z1")
                  nc.vector.tensor_reduce(
                      out=z[:], in_=w[:].rearrange("p k h -> p h k"),
                      axis=mybir.AxisListType.X, op=OP.add)
                  nc.vector.tensor_scalar_add(z[:], z[:], EPS)
                  rz = pb.tile([128, H1], F32, tag="rz1")
                  nc.vector.reciprocal(rz[:], z[:])
                  o = pb.tile([128, D1], F32, tag="o1")
                  nc.vector.tensor_tensor(
                      out=o[:].rearrange("p (h c) -> p h c", c=C1),
                      in0=S[:].rearrange("p (h c) -> p h c", c=C1),
                      in1=rz[:].unsqueeze(2).to_broadcast([128, H1, C1]),
                      op=OP.mult)
                  nc.vector.tensor_tensor(
                      out=o[:], in0=o[:], in1=b1_t[:], op=OP.add)
                  tmin = pb.tile([128, D1], F32, tag="tm1")
                  nc.vector.tensor_scalar_min(tmin[:], o[:], 0.0)
                  texp = pb.tile([128, D1], F32, tag="te1")
                  nc.scalar.activation(texp[:], tmin[:], AF.Exp)
                  nc.vector.tensor_scalar_max(o[:], o[:], 0.0)
                  nc.vector.scalar_tensor_tensor(
                      out=o1_st[:, t * D1:(t + 1) * D1], in0=texp[:],
                      scalar=-1.0, in1=o[:], op0=OP.add, op1=OP.add)
              nc.sync.dma_start(
                  out=o1_d.ap().rearrange("(t p) c -> p t c", p=128),
                  in_=o1_st[:].rearrange("p (t c) -> p t c", c=D1))

            # ---------------- phase A2: L2 projection ----------------
            with tc.tile_pool(name="pc", bufs=3) as pc, \
                 tc.tile_pool(name="pc1", bufs=1) as pc1, \
                 tc.tile_pool(name="psC", bufs=4, space="PSUM") as psC:
                id_t = pc1.tile([128, 128], F32)
                nc.sync.dma_start(out=id_t[:], in_=id_d[:])
                w2_t = pc1.tile([D1, C2 + 2], F32)
                nc.sync.dma_start(out=w2_t[:], in_=W2_d[:])
                st2 = pc1.tile([128, T, RW], BF16)
                nc.vector.memset(st2[:], 0.0)
                stA2 = pc1.tile([128, T], F32)
                for t in range(T):
                    h = pc.tile([128, D1], F32, tag="h1n")
                    nc.sync.dma_start(out=h[:], in_=o1_d[t * 128:(t + 1) * 128, :])
                    pst = psC.tile([D1, 128], F32, tag="psT")
                    nc.tensor.transpose(pst[:], h[:], id_t[:])
                    hT = pc.tile([D1, 128], F32, tag="hT")
                    nc.vector.tensor_copy(out=hT[:], in_=pst[:])
                    ps2 = psC.tile([128, C2 + 2], F32, tag="ps2")
                    nc.tensor.matmul(ps2[:], lhsT=hT[:], rhs=w2_t[:],
                                     start=True, stop=True)
                    nc.vector.tensor_copy(out=st2[:, t, 0:C2 + 1],
                                          in_=ps2[:, 0:C2 + 1])
                    nc.vector.tensor_copy(out=stA2[:, t:t + 1],
                                          in_=ps2[:, C2 + 1:C2 + 2])
                nc.sync.dma_start(
                    out=t2s_d.ap().rearrange("(t p) c -> p t c", p=128),
                    in_=st2[:])
                nc.sync.dma_start(
                    out=ad2_d.ap().rearrange("(t p) c -> p t c", p=128),
                    in_=stA2[:].unsqueeze(2))

            nc.gpsimd.collective_compute(
                "AllGather", OP.bypass, replica_groups=groups,
                ins=[t2s_d[:]], outs=[t2f_d[:]])

            with tc.tile_pool(name="poi2", bufs=1) as poi2:
                pz2 = poi2.tile([1, 1], BF16)
                nc.vector.memset(pz2[:], POISON)
                for c in range(4):
                    nc.sync.dma_start(
                        out=t2f_d[c * CH:c * CH + 1, C2:C2 + 1], in_=pz2[:])

            # ---------------- phase C: L2 edge + log_softmax ----------------
            with tc.tile_pool(name="pd", bufs=4) as pd, \
                 tc.tile_pool(name="pd1", bufs=1) as pd1:
              ad2_all = pd1.tile([128, T, 1], F32)
              nc.sync.dma_start(out=ad2_all[:],
                                in_=ad2_d.ap().rearrange("(t p) c -> p t c", p=128))
              o2_st = pd1.tile([128, T * C2], F32)
              b2_t = pd1.tile([128, C2], F32)
              nc.sync.dma_start(out=b2_t[:], in_=b2_d[:].to_broadcast([128, C2]))

              for t in range(T):
                  ent, K = sched[t], K2[t]
                  g2 = pd.tile([128, K, RW], BF16, tag="g2")
                  for (c, wl, io, go) in ent[0]:
                      nc.gpsimd.dma_gather(
                          out_ap=g2[:, go:go + wl, :],
                          in_ap=t2f_d[c * CH:(c + 1) * CH, :],
                          idxs_ap=idx_t[:, io:io + wl * 8],
                          num_idxs=wl * 128, num_idxs_reg=wl * 128,
                          elem_size=RW, queue_num=next_q())
                  ad2 = ad2_all[:, t, :]
                  as2 = pd.tile([128, K], F32, tag="as2")
                  nc.vector.tensor_copy(out=as2[:], in_=g2[:, :, C2])
                  lg2 = pd.tile([128, K], F32, tag="lg2")
                  nc.vector.tensor_tensor(
                      out=lg2[:], in0=as2[:],
                      in1=ad2.to_broadcast([128, K]), op=OP.add)
                  nc.vector.scalar_tensor_tensor(
                      out=lg2[:], in0=lg2[:], scalar=NEG_SLOPE, in1=lg2[:],
                      op0=OP.mult, op1=OP.max)
                  w2 = pd.tile([128, K], BF16, tag="w2")
                  nc.scalar.activation(w2[:], lg2[:], AF.Exp)
                  nc.vector.tensor_tensor(
                      out=g2[:, :, 0:C2], in0=g2[:, :, 0:C2],
                      in1=w2[:].unsqueeze(2).to_broadcast([128, K, C2]),
                      op=OP.mult)
                  S2 = pd.tile([128, C2], F32, tag="S2")
                  nc.vector.tensor_reduce(
                      out=S2[:], in_=g2[:, :, 0:C2].rearrange("p k c -> p c k"),
                      axis=mybir.AxisListType.X, op=OP.add)
                  z2 = pd.tile([128, 1], F32, tag="z2")
                  nc.vector.tensor_reduce(
                      out=z2[:], in_=w2[:].unsqueeze(1),
                      axis=mybir.AxisListType.X, op=OP.add)
                  nc.vector.tensor_scalar_add(z2[:], z2[:], EPS)
                  rz2 = pd.tile([128, 1], F32, tag="rz2")
                  nc.vector.reciprocal(rz2[:], z2[:])
                  o2 = pd.tile([128, C2], F32, tag="o2")
                  nc.vector.tensor_tensor(
                      out=o2[:], in0=S2[:],
                      in1=rz2[:].to_broadcast([128, C2]), op=OP.mult)
                  nc.vector.tensor_tensor(
                      out=o2[:], in0=o2[:], in1=b2_t[:], op=OP.add)
                  mx = pd.tile([128, 1], F32, tag="mx")
                  nc.vector.tensor_reduce(out=mx[:], in_=o2[:],
                                          axis=mybir.AxisListType.X, op=OP.max)
                  nc.vector.tensor_scalar(
                      out=o2[:], in0=o2[:], scalar1=mx[:], scalar2=None,
                      op0=OP.subtract)
                  ex = pd.tile([128, C2], F32, tag="ex")
                  nc.scalar.activation(ex[:], o2[:], AF.Exp)
                  sz = pd.tile([128, 1], F32, tag="sz")
                  nc.vector.tensor_reduce(out=sz[:], in_=ex[:],
                                          axis=mybir.AxisListType.X, op=OP.add)
                  lnz = pd.tile([128, 1], F32, tag="lnz")
                  nc.scalar.activation(lnz[:], sz[:], AF.Ln)
                  nc.vector.tensor_scalar(
                      out=o2_st[:, t * C2:(t + 1) * C2], in0=o2[:],
                      scalar1=lnz[:], scalar2=None, op0=OP.subtract)
              nc.sync.dma_start(
                  out=out_d.ap().rearrange("(t p) c -> p t c", p=128),
                  in_=o2_st[:].rearrange("p (t c) -> p t c", c=C2))

    nc.compile()
    return nc


# ------------------------------------------------------------- public entry

def _make_runner(nc, n_cores):
    """Jitted shard_map callable for a prebuilt Bass module (self-contained)."""
    import jax
    from jax.sharding import Mesh, PartitionSpec
    from jax.experimental.shard_map import shard_map
    from concourse import bass2jax
    from concourse.bass2jax import _bass_exec_p, partition_id_tensor

    bass2jax.install_neuronx_cc_hook()
    partition_name = nc.partition_id_tensor.name if nc.partition_id_tensor else None
    in_names, out_names, out_avals, zero_outs = [], [], [], []
    for alloc in nc.m.functions[0].allocations:
        if not isinstance(alloc, mybir.MemoryLocationSet):
            continue
        name = alloc.memorylocations[0].name
        if alloc.kind == "ExternalInput":
            if name != partition_name:
                in_names.append(name)
        elif alloc.kind == "ExternalOutput":
            out_names.append(name)
            shape = tuple(alloc.tensor_shape)
            dtype = mybir.dt.np(alloc.dtype)
            out_avals.append(jax.core.ShapedArray(shape, dtype))
            zero_outs.append(np.zeros(shape, dtype))
    n_params = len(in_names)
    n_outs = len(out_avals)
    all_in_names = list(in_names) + list(out_names)
    if partition_name is not None:
        all_in_names.append(partition_name)

    def _body(*args):
        operands = list(args)
        if partition_name is not None:
            operands.append(partition_id_tensor())
        outs = _bass_exec_p.bind(
            *operands,
            out_avals=tuple(out_avals),
            in_names=tuple(all_in_names),
            out_names=tuple(out_names),
            lowering_input_output_aliases=(),
            sim_require_finite=True,
            sim_require_nnan=True,
            nc=nc,
        )
        return tuple(outs)

    devices = jax.devices()[:n_cores]
    mesh = Mesh(np.asarray(devices), ("core",))
    in_specs = (PartitionSpec("core"),) * (n_params + n_outs)
    out_specs = (PartitionSpec("core"),) * n_outs
    sharded = jax.jit(
        shard_map(_body, mesh=mesh, in_specs=in_specs, out_specs=out_specs,
                  check_rep=False),
        keep_unused=True,
    )

    def stage(in_maps):
        import jax
        concat_in = [
            np.concatenate([np.asarray(in_maps[c][nm]) for c in range(n_cores)], 0)
            for nm in in_names
        ] + [np.concatenate([z] * n_cores, 0) for z in zero_outs]
        return [jax.device_put(a) for a in concat_in]

    return sharded, stage, out_names, out_avals



_CACHE = {}


def _runner_for(meta):
    key = (meta["NP"], meta["TOTW"], tuple(meta["K2"]))
    if key in _CACHE:
        return _CACHE[key]
    nc = build_program(meta)
    n_cores = meta["n_cores"]
    sharded, stage, out_names, out_avals = _make_runner(nc, n_cores)

    def run(in_maps):
        import jax
        staged = stage(in_maps)
        outs = sharded(*staged)
        jax.block_until_ready(outs)
        outs_np = [np.asarray(o) for o in outs]
        results = []
        for c in range(n_cores):
            m = {}
            for i, nm in enumerate(out_names):
                sh = out_avals[i].shape
                m[nm] = outs_np[i][c * sh[0]:(c + 1) * sh[0]]
            results.append(m)
        return results

    _CACHE[key] = run
    return run


def kernel(x, edge_index, W1, att_src1, att_dst1, b1, W2, att_src2,
           att_dst2, b2):
    """Full-input GAT forward on 8 NeuronCores; returns [N, C2] float32."""
    x = np.asarray(x)
    N = x.shape[0]
    in_maps, meta, node_lists = host_prep(
        x, edge_index, W1, att_src1, att_dst1, b1, W2, att_src2, att_dst2,
        b2, n_cores=8)
    run = _runner_for(meta)
    results = run(in_maps)
    C2 = meta["C2"]
    nat = np.empty((meta["NP"], C2), np.float32)
    for c in range(meta["n_cores"]):
        nat[node_lists[c]] = results[c]["out"]
    return np.ascontiguousarray(nat[:N], dtype=np.float32)
